# revision 1
# baseline (speedup 1.0000x reference)
"""3-layer GCN node classifier on 8 Trainium2 NeuronCores.

Math (per layer, folding the symmetric normalization):
    deg[v]  = in-degree(v) (with self loop), dinv = rsqrt(deg)
    g       = dinv * (h @ W)                  (rows scaled)
    out[c]  = dinv[c] * ( sum_{e: col=c} g[row_e] + g[c] ) + b
    h_next  = relu(out)      (layers 1,2; layer 3 has no relu)

Distribution: nodes are range-sharded across 8 cores (graph parallel).
Each core computes g for its own nodes (dense matmul), the g-table is
all-gathered to every core's HBM, each core then gathers the rows for
the edges whose *destination* it owns (SWDGE dma_gather) and
scatter-adds them into per-destination-tile PSUM accumulators via
one-hot matmuls on the TensorEngine.

Host-side preprocessing only touches edge_index (graph structure):
CSR-style bucketing of edges by destination tile, degree computation,
a within-core node permutation that load-balances destination tiles,
and int16 gather-index packing (the 50k-row table is split into two
overlapping <=32768-row views because SWDGE gather indices are int16).
"""

import math
import os
import numpy as np

# ---------------------------------------------------------------------------
# problem constants (hardcoded per contract; kernel.py must be self-contained)
# ---------------------------------------------------------------------------
N_NODES = 50000
IN_C, HID_C, OUT_C = 128, 128, 64
M_CORES = 8
NPC = N_NODES // M_CORES            # 6250 nodes per core
TPC = (NPC + 127) // 128            # 49 destination tiles per core
NPAD = TPC * 128                    # 6272 padded nodes per core
TBL = M_CORES * NPAD                # 50176 rows in the all-gathered table
LO_CAP = 32768                      # int16 index reach
HI_OFF = max(0, TBL - 32768)        # 17408: hi view = table[HI_OFF:]

F32 = "float32"


# ---------------------------------------------------------------------------
# host-side graph preprocessing (indices only)
# ---------------------------------------------------------------------------
def _plan(edge_index: np.ndarray):
    """Build per-core index/metadata arrays from edge_index [2, E]."""
    row = np.asarray(edge_index[0], dtype=np.int64)
    col = np.asarray(edge_index[1], dtype=np.int64)

    deg_in = np.bincount(col, minlength=N_NODES)          # edges only
    dinv = 1.0 / np.sqrt(deg_in + 1.0)                     # + self loop

    # within-core permutation: snake-fill tiles with degree-sorted nodes so
    # every destination tile carries a near-equal number of incoming edges.
    pos_local = np.empty(N_NODES, dtype=np.int64)
    for k in range(M_CORES):
        v0 = k * NPC
        d = deg_in[v0 : v0 + NPC]
        order = np.argsort(-d, kind="stable")              # heavy first
        # serpentine tile ids: 0..T-1, T-1..0, ...
        idx = np.arange(NPC)
        rnd, off = divmod(idx, TPC)
        tile_ids = np.where(rnd % 2 == 0, off, TPC - 1 - off)
        slot_in_tile = rnd
        pos = tile_ids * 128 + slot_in_tile
        pos_local[v0 + order] = pos

    g_pos = (np.arange(N_NODES) // NPC) * NPAD + pos_local  # table row per node

    # per-core per-tile edge buckets
    kd = col // NPC
    src_pos = g_pos[row]
    dst_slot = pos_local[col]
    dst_tile = dst_slot // 128
    dst_loc = dst_slot % 128
    is_lo = src_pos < LO_CAP

    # counts to size K_lo / K_hi uniformly across the SPMD program
    tile_key = kd * TPC + dst_tile
    n_lo = np.bincount(tile_key[is_lo], minlength=M_CORES * TPC)
    n_hi = np.bincount(tile_key[~is_lo], minlength=M_CORES * TPC)
    K_lo = max(1, int(math.ceil(n_lo.max() / 128)))
    K_hi = max(1, int(math.ceil(n_hi.max() / 128)))
    K = K_lo + K_hi
    NCH = TPC * K

    per_core = []
    for k in range(M_CORES):
        idx16 = np.zeros((TPC, K, 128), dtype=np.int16)    # pad -> row 0
        dloc_a = np.full((TPC, K, 128), 200.0, dtype=np.float32)
        m = kd == k
        tl, lc, sp, lo = dst_tile[m], dst_loc[m], src_pos[m], is_lo[m]
        for t in range(TPC):
            mt = tl == t
            for stream, base in ((lo & mt, 0), ((~lo) & mt, K_lo)):
                sps = sp[stream]
                lcs = lc[stream]
                n = sps.size
                if base == 0:
                    vals = sps
                else:
                    vals = sps - HI_OFF
                flat_i = idx16[t].reshape(-1)
                flat_d = dloc_a[t].reshape(-1)
                o = base * 128
                flat_i[o : o + n] = vals.astype(np.int16)
                flat_d[o : o + n] = lcs.astype(np.float32)

        # SWDGE wrapped-16 index layout, replicated across the 8 Q7 groups
        flat = idx16.reshape(-1)
        wrapped = flat.reshape(-1, 16).T                    # [16, NCH*8]
        idx_sb = np.tile(wrapped, (8, 1)).copy()            # [128, NCH*8]
        # per-chunk destination-local column, lane-major
        dloc_sb = dloc_a.transpose(2, 0, 1).reshape(128, NCH).copy()
        # per-slot dinv (0 on dummy slots)
        dv = np.zeros(NPAD, dtype=np.float32)
        v0 = k * NPC
        dv[pos_local[v0 : v0 + NPC]] = dinv[v0 : v0 + NPC]
        dinv_sb = dv.reshape(TPC, 128).T.copy()             # [128, TPC]
        per_core.append(dict(idx=idx_sb, dloc=dloc_sb, dinv=dinv_sb))

    return dict(
        K_lo=K_lo, K_hi=K_hi, NCH=NCH, per_core=per_core,
        pos_local=pos_local, dinv=dinv,
    )


# ---------------------------------------------------------------------------
# device program
# ---------------------------------------------------------------------------
def _build_nc(K_lo: int, K_hi: int, with_bias: bool, reps: int = 0, ablate: str = ''):
    abl = set(a for a in ablate.split(',') if a)
    """Build + compile the SPMD program.

    reps > 0 additionally emits a timing loop: the full pipeline runs once
    (correct, fills the gful tables), then a hardware For_i loop re-runs
    the whole body `reps` times with the collectives elided (collectives
    cannot sit inside control flow) so device time dominates wall clock.
    """
    import concourse.bacc as bacc
    import concourse.mybir as mybir
    from concourse import tile
    from concourse._compat import get_trn_type

    dt = mybir.dt
    K = K_lo + K_hi
    NCH = TPC * K
    NW = NCH * 8

    nc = bacc.Bacc(
        get_trn_type() or "TRN2",
        target_bir_lowering=False,
        debug=False,
        enable_asserts=False,
        num_devices=M_CORES,
    )

    # I/O
    xT_p = nc.dram_tensor("xT", [128, NPAD], dt.float32, kind="ExternalInput")
    W1_p = nc.dram_tensor("W1", [IN_C, HID_C], dt.float32, kind="ExternalInput")
    W2_p = nc.dram_tensor("W2", [HID_C, HID_C], dt.float32, kind="ExternalInput")
    W3_p = nc.dram_tensor("W3", [HID_C, OUT_C], dt.float32, kind="ExternalInput")
    dinv_p = nc.dram_tensor("dinv", [128, TPC], dt.float32, kind="ExternalInput")
    dloc_p = nc.dram_tensor("dloc", [128, NCH], dt.float32, kind="ExternalInput")
    idx_p = nc.dram_tensor("idx", [128, NW], dt.int16, kind="ExternalInput")
    iota_p = nc.dram_tensor("iota", [128, 128], dt.float32, kind="ExternalInput")
    ident_p = nc.dram_tensor("ident", [128, 128], dt.float32, kind="ExternalInput")
    if with_bias:
        b1_p = nc.dram_tensor("b1r", [128, HID_C], dt.float32, kind="ExternalInput")
        b2_p = nc.dram_tensor("b2r", [128, HID_C], dt.float32, kind="ExternalInput")
        b3_p = nc.dram_tensor("b3r", [128, OUT_C], dt.float32, kind="ExternalInput")
    out_p = nc.dram_tensor("out", [NPAD, OUT_C], dt.float32, kind="ExternalOutput")

    RG = [list(range(M_CORES))]
    AF = mybir.ActivationFunctionType
    OP = mybir.AluOpType

    with tile.TileContext(nc) as tc, tc.tile_pool(name="persist", bufs=1) as pp:
        # persistent SBUF tiles (one slot each)
        hT_a = pp.tile([128, NPAD], dt.float32, name="hT_a")
        hT_b = pp.tile([128, NPAD], dt.float32, name="hT_b")
        w1_sb = pp.tile([128, HID_C], dt.float32, name="w1_sb")
        w2_sb = pp.tile([128, HID_C], dt.float32, name="w2_sb")
        w3_sb = pp.tile([128, OUT_C], dt.float32, name="w3_sb")
        dinv_sb = pp.tile([128, TPC], dt.float32, name="dinv_sb")
        dloc_sb = pp.tile([128, NCH], dt.float32, name="dloc_sb")
        idx_sb = pp.tile([128, NW], dt.int16, name="idx_sb")
        iota_sb = pp.tile([128, 128], dt.float32, name="iota_sb")
        ident_sb = pp.tile([128, 128], dt.float32, name="ident_sb")
        bias_sb = []

        nc.sync.dma_start(hT_a[:], xT_p[:])
        nc.sync.dma_start(w1_sb[:], W1_p[:])
        nc.sync.dma_start(w2_sb[:], W2_p[:])
        nc.sync.dma_start(w3_sb[:], W3_p[:])
        nc.sync.dma_start(dinv_sb[:], dinv_p[:])
        nc.sync.dma_start(dloc_sb[:], dloc_p[:])
        nc.sync.dma_start(idx_sb[:], idx_p[:])
        nc.sync.dma_start(iota_sb[:], iota_p[:])
        nc.sync.dma_start(ident_sb[:], ident_p[:])
        if with_bias:
            for p, cc in ((b1_p, HID_C), (b2_p, HID_C), (b3_p, OUT_C)):
                t = pp.tile([128, cc], dt.float32, name=f"bias{len(bias_sb)}_sb")
                nc.sync.dma_start(t[:], p[:])
                bias_sb.append(t)

        layers = [
            (w1_sb, HID_C, True, hT_a, hT_b),
            (w2_sb, HID_C, True, hT_b, hT_a),
            (w3_sb, OUT_C, False, hT_a, None),
        ]

        with (
            tc.tile_pool(name="gsb", bufs=2) as gsb_pool,
            tc.tile_pool(name="msg", bufs=4) as msg_pool,
            tc.tile_pool(name="oh", bufs=4) as oh_pool,
            tc.tile_pool(name="eps", bufs=3) as eps_pool,
            tc.tile_pool(name="psA", bufs=2, space="PSUM") as psA_pool,
            tc.tile_pool(name="psS", bufs=2, space="PSUM") as psS_pool,
            tc.tile_pool(name="psT", bufs=2, space="PSUM") as psT_pool,
            tc.tile_pool(name="dram", bufs=1, space="DRAM") as dram_pool,
        ):
            glocs = [
                dram_pool.tile([NPAD, c], dt.float32, name=f"gloc{i}")
                for i, c in enumerate([HID_C, HID_C, OUT_C])
            ]
            gfuls = [
                dram_pool.tile(
                    [TBL, c], dt.float32,
                    addr_space="Shared" if M_CORES > 4 else "Local",
                    name=f"gful{i}",
                )
                for i, c in enumerate([HID_C, HID_C, OUT_C])
            ]

            def emit_layers(with_cc):
                for li, (w_sb, C, relu, hT_in, hT_out) in enumerate(layers):
                    gloc, gful = glocs[li], gfuls[li]

                    # stage A: g = dinv * (h @ W) for own nodes
                    g_sb = gsb_pool.tile([128, TPC, C], dt.float32, tag="gsb")
                    for t in range(TPC):
                        psA = psA_pool.tile([128, C], dt.float32, tag="psA")
                        nc.tensor.matmul(
                            psA[:],
                            lhsT=hT_in[:, t * 128 : (t + 1) * 128],
                            rhs=w_sb[:, :C],
                            start=True,
                            stop=True,
                        )
                        nc.vector.tensor_scalar_mul(
                            g_sb[:, t, :], psA[:], dinv_sb[:, t : t + 1]
                        )
                    nc.sync.dma_start(
                        gloc[:].rearrange("(t p) c -> p t c", p=128), g_sb[:]
                    )

                    # stage B: replicate the g table
                    if with_cc:
                        nc.gpsimd.collective_compute(
                            "AllGather",
                            OP.bypass,
                            replica_groups=RG,
                            ins=[gloc[:]],
                            outs=[gful[:]],
                        )
                    g_lo = gful[0 : min(LO_CAP, TBL), :]
                    g_hi = gful[HI_OFF:TBL, :]

                    # stage C: gather + one-hot scatter per destination tile
                    Ce = 64 if 'e64' in abl else C
                    for t in range(TPC):
                        woff = t * K * 8
                        if 'nogather' not in abl:
                            msg = msg_pool.tile([128, K, Ce], dt.float32, tag="msg")
                            nc.gpsimd.dma_gather(
                                msg[:, 0:K_lo, :],
                                g_lo[:, :Ce],
                                idx_sb[:, woff : woff + K_lo * 8],
                                K_lo * 128,
                                K_lo * 128,
                                Ce,
                                elem_step=C,
                                single_packet=False,
                            )
                            nc.gpsimd.dma_gather(
                                msg[:, K_lo:K, :],
                                g_hi[:, :Ce],
                                idx_sb[:, woff + K_lo * 8 : woff + K * 8],
                                K_hi * 128,
                                K_hi * 128,
                                Ce,
                                elem_step=C,
                                single_packet=False,
                            )
                        psS = psS_pool.tile([128, Ce], dt.float32, tag="psS")
                        n_mm = 1 if 'nomm' in abl else K
                        for j in range(n_mm):
                            if 'nooh' in abl:
                                mm_lhs = ident_sb
                            else:
                                oh = oh_pool.tile([128, 128], dt.float32, tag="oh")
                                nc.vector.tensor_scalar(
                                    oh[:],
                                    iota_sb[:],
                                    dloc_sb[:, t * K + j : t * K + j + 1],
                                    None,
                                    op0=OP.is_equal,
                                )
                                mm_lhs = oh
                            mm_rhs = (
                                msg[:, j, :]
                                if 'nogather' not in abl
                                else hT_a[:, j * 64 : j * 64 + Ce]
                            )
                            nc.tensor.matmul(
                                psS[:],
                                lhsT=mm_lhs[:],
                                rhs=mm_rhs,
                                start=(j == 0),
                                stop=(j == n_mm - 1),
                            )
                        # epilogue: + self-loop, * dinv, (+bias), relu
                        acc = eps_pool.tile([128, C], dt.float32, tag="acc")
                        nc.vector.tensor_add(
                            acc[:, :Ce], psS[:], g_sb[:, t, :Ce]
                        )
                        if Ce < C:
                            nc.vector.tensor_copy(acc[:, Ce:], g_sb[:, t, Ce:])
                        h_t = eps_pool.tile([128, C], dt.float32, tag="h_t")
                        if with_bias:
                            nc.vector.tensor_scalar_mul(
                                acc[:], acc[:], dinv_sb[:, t : t + 1]
                            )
                            nc.vector.tensor_add(acc[:], acc[:], bias_sb[li][:])
                            if relu:
                                nc.scalar.activation(h_t[:], acc[:], AF.Relu)
                            else:
                                nc.scalar.copy(h_t[:], acc[:])
                        else:
                            if relu:
                                nc.scalar.activation(
                                    h_t[:], acc[:], AF.Relu,
                                    scale=dinv_sb[:, t : t + 1],
                                )
                            else:
                                nc.scalar.mul(
                                    h_t[:], acc[:], dinv_sb[:, t : t + 1]
                                )
                        if hT_out is not None:
                            psT = psT_pool.tile([128, 128], dt.float32, tag="psT")
                            nc.tensor.transpose(psT[:], h_t[:], ident_sb[:])
                            nc.vector.tensor_copy(
                                hT_out[:, t * 128 : (t + 1) * 128], psT[:]
                            )
                        else:
                            nc.sync.dma_start(
                                out_p[t * 128 : (t + 1) * 128, :], h_t[:]
                            )

            emit_layers(with_cc=True)
            if reps:
                with tc.For_i(0, reps, 1):
                    emit_layers(with_cc=False)

    nc.compile()
    return nc


_NC_CACHE: dict = {}


def _get_nc(K_lo, K_hi, with_bias):
    key = (K_lo, K_hi, with_bias)
    if key not in _NC_CACHE:
        _NC_CACHE[key] = _build_nc(K_lo, K_hi, with_bias)
    return _NC_CACHE[key]


# ---------------------------------------------------------------------------
# entry point
# ---------------------------------------------------------------------------
def _prepare(x, edge_index, W1, b1, W2, b2, W3, b3):
    x = np.asarray(x, dtype=np.float32)
    W1 = np.asarray(W1, dtype=np.float32)
    W2 = np.asarray(W2, dtype=np.float32)
    W3 = np.asarray(W3, dtype=np.float32)
    b1 = np.asarray(b1, dtype=np.float32)
    b2 = np.asarray(b2, dtype=np.float32)
    b3 = np.asarray(b3, dtype=np.float32)

    plan = _plan(np.asarray(edge_index))
    with_bias = bool(np.any(b1) or np.any(b2) or np.any(b3))
    nc = _get_nc(plan["K_lo"], plan["K_hi"], with_bias)

    iota = np.tile(np.arange(128, dtype=np.float32), (128, 1))
    ident = np.eye(128, dtype=np.float32)
    pos_local = plan["pos_local"]

    in_maps = []
    for k in range(M_CORES):
        pc = plan["per_core"][k]
        v0 = k * NPC
        xT = np.zeros((128, NPAD), dtype=np.float32)
        xT[:, pos_local[v0 : v0 + NPC]] = x[v0 : v0 + NPC].T
        im = dict(
            xT=xT, W1=W1, W2=W2, W3=W3,
            dinv=pc["dinv"], dloc=pc["dloc"], idx=pc["idx"],
            iota=iota, ident=ident,
        )
        if with_bias:
            im["b1r"] = np.tile(b1, (128, 1)).astype(np.float32)
            im["b2r"] = np.tile(b2, (128, 1)).astype(np.float32)
            im["b3r"] = np.tile(b3, (128, 1)).astype(np.float32)
        in_maps.append(im)

    def unpermute(results):
        out = np.empty((N_NODES, OUT_C), dtype=np.float32)
        for k in range(M_CORES):
            v0 = k * NPC
            r = results[k]["out"]
            out[v0 : v0 + NPC] = r[pos_local[v0 : v0 + NPC]]
        return out

    return nc, in_maps, unpermute


def kernel(x, edge_index, W1, b1, W2, b2, W3, b3):
    from concourse.bass_utils import run_bass_kernel_spmd

    nc, in_maps, unpermute = _prepare(x, edge_index, W1, b1, W2, b2, W3, b3)
    res = run_bass_kernel_spmd(nc, in_maps, list(range(M_CORES)))
    return unpermute(res.results)



# revision 6
# speedup vs baseline: 1.1464x; 1.1464x over previous
"""3-layer GCN node classifier on 8 Trainium2 NeuronCores.

Math (per layer, folding the symmetric normalization):
    deg[v]  = in-degree(v) (with self loop), dinv = rsqrt(deg)
    g       = dinv * (h @ W)                  (rows scaled)
    out[c]  = dinv[c] * ( sum_{e: col=c} g[row_e] + g[c] ) + b
    h_next  = relu(out)      (layers 1,2; layer 3 has no relu)

Distribution: nodes are range-sharded across 8 cores (graph parallel).
Each core computes g for its own nodes (dense matmul), the g-table is
all-gathered to every core's HBM in bf16, each core then gathers the
rows for the edges whose *destination* it owns (SWDGE dma_gather over
4 queues, merged across tile blocks) and scatter-adds them into
per-destination-tile PSUM accumulators via one-hot matmuls (bf16) on
the TensorEngine.

Host-side preprocessing only touches edge_index (graph structure):
CSR-style bucketing of edges by destination tile, degree computation,
a within-core node permutation that load-balances destination tiles,
ascending-address sorting of gather indices, and int16 gather-index
packing (the 50k-row table is split into two overlapping <=32768-row
views because SWDGE gather indices are int16).
"""

import math
import os
import numpy as np

# ---------------------------------------------------------------------------
# problem constants (hardcoded per contract; kernel.py must be self-contained)
# ---------------------------------------------------------------------------
N_NODES = 50000
IN_C, HID_C, OUT_C = 128, 128, 64
M_CORES = 8
NPC = N_NODES // M_CORES            # 6250 nodes per core
TPC = (NPC + 127) // 128            # 49 destination tiles per core
NPAD = TPC * 128                    # 6272 padded nodes per core
TBL = M_CORES * NPAD                # 50176 rows in the all-gathered table
LO_CAP = 32768                      # int16 index reach
HI_OFF = max(0, TBL - 32768)        # 17408: hi view = table[HI_OFF:]
NTB = 7                             # destination tiles per gather block
NBLK = TPC // NTB                   # 7 blocks per core

F32 = "float32"


# ---------------------------------------------------------------------------
# host-side graph preprocessing (indices only)
# ---------------------------------------------------------------------------
def _plan(edge_index: np.ndarray):
    """Build per-core index/metadata arrays from edge_index [2, E]."""
    row = np.asarray(edge_index[0], dtype=np.int64)
    col = np.asarray(edge_index[1], dtype=np.int64)

    deg_in = np.bincount(col, minlength=N_NODES)          # edges only
    dinv = 1.0 / np.sqrt(deg_in + 1.0)                     # + self loop

    # within-core permutation: snake-fill tiles with degree-sorted nodes so
    # every destination tile carries a near-equal number of incoming edges.
    pos_local = np.empty(N_NODES, dtype=np.int64)
    for k in range(M_CORES):
        v0 = k * NPC
        d = deg_in[v0 : v0 + NPC]
        order = np.argsort(-d, kind="stable")              # heavy first
        # serpentine tile ids: 0..T-1, T-1..0, ...
        idx = np.arange(NPC)
        rnd, off = divmod(idx, TPC)
        tile_ids = np.where(rnd % 2 == 0, off, TPC - 1 - off)
        slot_in_tile = rnd
        pos = tile_ids * 128 + slot_in_tile
        pos_local[v0 + order] = pos

    g_pos = (np.arange(N_NODES) // NPC) * NPAD + pos_local  # table row per node

    # per-core per-tile edge buckets
    kd = col // NPC
    src_pos = g_pos[row]
    dst_slot = pos_local[col]
    dst_tile = dst_slot // 128
    dst_loc = dst_slot % 128
    is_lo = src_pos < LO_CAP

    # counts to size K_lo / K_hi uniformly across the SPMD program
    tile_key = kd * TPC + dst_tile
    n_lo = np.bincount(tile_key[is_lo], minlength=M_CORES * TPC)
    n_hi = np.bincount(tile_key[~is_lo], minlength=M_CORES * TPC)
    K_lo = max(1, int(math.ceil(n_lo.max() / 128)))
    K_hi = max(1, int(math.ceil(n_hi.max() / 128)))
    K = K_lo + K_hi
    NCH = TPC * K

    per_core = []
    for k in range(M_CORES):
        # global chunk order: per block, all lo chunks (NTB*K_lo) then all
        # hi chunks (NTB*K_hi); within a tile's stream, indices sorted
        # ascending (HBM locality).
        idx16 = np.zeros((NCH, 128), dtype=np.int16)       # pad -> row 0
        dloc_a = np.full((NCH, 128), 200.0, dtype=np.float32)
        m = kd == k
        tl, lc, sp, lo = dst_tile[m], dst_loc[m], src_pos[m], is_lo[m]
        for b in range(NBLK):
            boff = b * NTB * K
            for tloc in range(NTB):
                t = b * NTB + tloc
                mt = tl == t
                for stream, coff, kn, voff in (
                    (lo & mt, boff + tloc * K_lo, K_lo, 0),
                    ((~lo) & mt, boff + NTB * K_lo + tloc * K_hi, K_hi, HI_OFF),
                ):
                    sps = sp[stream]
                    o = np.argsort(sps, kind="stable")
                    sps = sps[o] - voff
                    lcs = lc[stream][o]
                    n = sps.size
                    fi = idx16[coff : coff + kn].reshape(-1)
                    fd = dloc_a[coff : coff + kn].reshape(-1)
                    fi[:n] = sps.astype(np.int16)
                    fd[:n] = lcs.astype(np.float32)

        # SWDGE wrapped-16 index layout, replicated across the 8 Q7 groups
        flat = idx16.reshape(-1)
        wrapped = flat.reshape(-1, 16).T                    # [16, NCH*8]
        idx_sb = np.tile(wrapped, (8, 1)).copy()            # [128, NCH*8]
        # per-chunk destination-local column, lane-major
        dloc_sb = dloc_a.T.copy()                           # [128, NCH]
        # per-slot dinv (0 on dummy slots)
        dv = np.zeros(NPAD, dtype=np.float32)
        v0 = k * NPC
        dv[pos_local[v0 : v0 + NPC]] = dinv[v0 : v0 + NPC]
        dinv_sb = dv.reshape(TPC, 128).T.copy()             # [128, TPC]
        per_core.append(dict(idx=idx_sb, dloc=dloc_sb, dinv=dinv_sb))

    return dict(
        K_lo=K_lo, K_hi=K_hi, NCH=NCH, per_core=per_core,
        pos_local=pos_local, dinv=dinv,
    )


# ---------------------------------------------------------------------------
# device program
# ---------------------------------------------------------------------------
def _build_nc(K_lo: int, K_hi: int, with_bias: bool, reps: int = 0, ablate: str = '',
              nq: int = 4, msg_bufs: int = 3):
    abl = set(a for a in ablate.split(',') if a)
    """Build + compile the SPMD program.

    reps > 0 additionally emits a timing loop: the full pipeline runs once
    (correct, fills the gful tables), then a hardware For_i loop re-runs
    the whole body `reps` times with the collectives elided (collectives
    cannot sit inside control flow) so device time dominates wall clock.
    """
    import concourse.bacc as bacc
    import concourse.mybir as mybir
    from concourse import tile
    from concourse._compat import get_trn_type

    dt = mybir.dt
    K = K_lo + K_hi
    NCH = TPC * K
    NW = NCH * 8

    nc = bacc.Bacc(
        get_trn_type() or "TRN2",
        target_bir_lowering=False,
        debug=False,
        enable_asserts=False,
        num_devices=M_CORES,
        num_swdge_queues=nq,
    )

    # I/O (xT/W in bf16; dinv/dloc fp32)
    xT_p = nc.dram_tensor("xT", [128, NPAD], dt.bfloat16, kind="ExternalInput")
    W1_p = nc.dram_tensor("W1", [IN_C, HID_C], dt.bfloat16, kind="ExternalInput")
    W2_p = nc.dram_tensor("W2", [HID_C, HID_C], dt.bfloat16, kind="ExternalInput")
    W3_p = nc.dram_tensor("W3", [HID_C, OUT_C], dt.bfloat16, kind="ExternalInput")
    dinv_p = nc.dram_tensor("dinv", [128, TPC], dt.float32, kind="ExternalInput")
    dloc_p = nc.dram_tensor("dloc", [128, NCH], dt.float32, kind="ExternalInput")
    idx_p = nc.dram_tensor("idx", [128, NW], dt.int16, kind="ExternalInput")
    iota_p = nc.dram_tensor("iota", [128, 128], dt.bfloat16, kind="ExternalInput")
    ident_p = nc.dram_tensor("ident", [128, 128], dt.bfloat16, kind="ExternalInput")
    if with_bias:
        b1_p = nc.dram_tensor("b1r", [128, HID_C], dt.float32, kind="ExternalInput")
        b2_p = nc.dram_tensor("b2r", [128, HID_C], dt.float32, kind="ExternalInput")
        b3_p = nc.dram_tensor("b3r", [128, OUT_C], dt.float32, kind="ExternalInput")
    out_p = nc.dram_tensor("out", [NPAD, OUT_C], dt.float32, kind="ExternalOutput")

    RG = [list(range(M_CORES))]
    AF = mybir.ActivationFunctionType
    OP = mybir.AluOpType

    with tile.TileContext(nc) as tc, tc.tile_pool(name="persist", bufs=1) as pp:
        # persistent SBUF tiles (one slot each)
        hT_a = pp.tile([128, NPAD], dt.bfloat16, name="hT_a")
        hT_b = pp.tile([128, NPAD], dt.bfloat16, name="hT_b")
        w1_sb = pp.tile([128, HID_C], dt.bfloat16, name="w1_sb")
        w2_sb = pp.tile([128, HID_C], dt.bfloat16, name="w2_sb")
        w3_sb = pp.tile([128, OUT_C], dt.bfloat16, name="w3_sb")
        dinv_sb = pp.tile([128, TPC], dt.float32, name="dinv_sb")
        dloc_sb = pp.tile([128, NCH], dt.float32, name="dloc_sb")
        idx_sb = pp.tile([128, NW], dt.int16, name="idx_sb")
        iota_sb = pp.tile([128, 128], dt.bfloat16, name="iota_sb")
        ident_sb = pp.tile([128, 128], dt.bfloat16, name="ident_sb")
        bias_sb = []

        nc.sync.dma_start(hT_a[:], xT_p[:])
        nc.sync.dma_start(w1_sb[:], W1_p[:])
        nc.sync.dma_start(w2_sb[:], W2_p[:])
        nc.sync.dma_start(w3_sb[:], W3_p[:])
        nc.sync.dma_start(dinv_sb[:], dinv_p[:])
        nc.sync.dma_start(dloc_sb[:], dloc_p[:])
        nc.sync.dma_start(idx_sb[:], idx_p[:])
        nc.sync.dma_start(iota_sb[:], iota_p[:])
        nc.sync.dma_start(ident_sb[:], ident_p[:])
        if with_bias:
            for p, cc in ((b1_p, HID_C), (b2_p, HID_C), (b3_p, OUT_C)):
                t = pp.tile([128, cc], dt.float32, name=f"bias{len(bias_sb)}_sb")
                nc.sync.dma_start(t[:], p[:])
                bias_sb.append(t)

        layers = [
            (w1_sb, HID_C, True, hT_a, hT_b),
            (w2_sb, HID_C, True, hT_b, hT_a),
            (w3_sb, OUT_C, False, hT_a, None),
        ]

        with (
            tc.tile_pool(name="gsb", bufs=2) as gsb_pool,
            tc.tile_pool(name="msg", bufs=msg_bufs) as msg_pool,
            tc.tile_pool(name="oh", bufs=4) as oh_pool,
            tc.tile_pool(name="eps", bufs=3) as eps_pool,
            tc.tile_pool(name="psA", bufs=2, space="PSUM") as psA_pool,
            tc.tile_pool(name="psS", bufs=2, space="PSUM") as psS_pool,
            tc.tile_pool(name="psT", bufs=2, space="PSUM") as psT_pool,
            tc.tile_pool(name="dram", bufs=1, space="DRAM") as dram_pool,
        ):
            # tables are bf16, layer-3 table padded to 128 cols (upper half
            # junk, never read by compute: matmul rhs slices cols 0:C)
            glocs = [
                dram_pool.tile([NPAD, 128], dt.bfloat16, name=f"gloc{i}")
                for i in range(3)
            ]
            gfuls = [
                dram_pool.tile(
                    [TBL, 128], dt.bfloat16,
                    addr_space="Shared" if M_CORES > 4 else "Local",
                    name=f"gful{i}",
                )
                for i in range(3)
            ]

            def emit_layers(with_cc):
                for li, (w_sb, C, relu, hT_in, hT_out) in enumerate(layers):
                    gloc, gful = glocs[li], gfuls[li]

                    # stage A: g = dinv * (h @ W) for own nodes
                    g_sb = gsb_pool.tile([128, TPC, 128], dt.bfloat16, tag="gsb")
                    for t in range(TPC):
                        psA = psA_pool.tile([128, C], dt.float32, tag="psA")
                        nc.tensor.matmul(
                            psA[:],
                            lhsT=hT_in[:, t * 128 : (t + 1) * 128],
                            rhs=w_sb[:, :C],
                            start=True,
                            stop=True,
                        )
                        nc.vector.tensor_scalar_mul(
                            g_sb[:, t, :C], psA[:], dinv_sb[:, t : t + 1]
                        )
                    nc.sync.dma_start(
                        gloc[:].rearrange("(t p) c -> p t c", p=128), g_sb[:]
                    )

                    # stage B: replicate the g table
                    if with_cc:
                        nc.gpsimd.collective_compute(
                            "AllGather",
                            OP.bypass,
                            replica_groups=RG,
                            ins=[gloc[:]],
                            outs=[gful[:]],
                        )
                    g_lo = gful[0 : min(LO_CAP, TBL), :]
                    g_hi = gful[HI_OFF:TBL, :]

                    # stage C: merged gather per block + one-hot scatter per
                    # destination tile
                    for b in range(NBLK):
                        boff = b * NTB * K
                        if 'nogather' not in abl:
                            msg = msg_pool.tile(
                                [128, NTB * K, 128], dt.bfloat16, tag="msg"
                            )
                            nc.gpsimd.dma_gather(
                                msg[:, 0 : NTB * K_lo, :],
                                g_lo[:],
                                idx_sb[:, boff * 8 : (boff + NTB * K_lo) * 8],
                                NTB * K_lo * 128,
                                NTB * K_lo * 128,
                                128,
                                elem_step=128,
                                single_packet=False,
                                queue_num=(2 * b) % nq,
                            )
                            nc.gpsimd.dma_gather(
                                msg[:, NTB * K_lo : NTB * K, :],
                                g_hi[:],
                                idx_sb[:, (boff + NTB * K_lo) * 8 : (boff + NTB * K) * 8],
                                NTB * K_hi * 128,
                                NTB * K_hi * 128,
                                128,
                                elem_step=128,
                                single_packet=False,
                                queue_num=(2 * b + 1) % nq,
                            )
                        for tloc in range(NTB):
                            t = b * NTB + tloc
                            psS = psS_pool.tile([128, C], dt.float32, tag="psS")
                            chunks = (
                                [(boff + tloc * K_lo + j) for j in range(K_lo)]
                                + [(boff + NTB * K_lo + tloc * K_hi + j) for j in range(K_hi)]
                            )
                            if 'nomm' in abl:
                                chunks = chunks[:1]
                            for jj, ch in enumerate(chunks):
                                if 'nooh' in abl:
                                    mm_lhs = ident_sb
                                else:
                                    oh = oh_pool.tile([128, 128], dt.bfloat16, tag="oh")
                                    nc.vector.tensor_scalar(
                                        oh[:],
                                        iota_sb[:],
                                        dloc_sb[:, ch : ch + 1],
                                        None,
                                        op0=OP.is_equal,
                                    )
                                    mm_lhs = oh
                                mm_rhs = (
                                    msg[:, ch - boff, :C]
                                    if 'nogather' not in abl
                                    else hT_a[:, jj * 64 : jj * 64 + C]
                                )
                                nc.tensor.matmul(
                                    psS[:],
                                    lhsT=mm_lhs[:],
                                    rhs=mm_rhs,
                                    start=(jj == 0),
                                    stop=(jj == len(chunks) - 1),
                                )
                            # epilogue: + self-loop, * dinv, (+bias), relu
                            acc = eps_pool.tile([128, C], dt.float32, tag="acc")
                            nc.vector.tensor_add(
                                acc[:], psS[:], g_sb[:, t, :C]
                            )
                            h_t = eps_pool.tile(
                                [128, C],
                                dt.bfloat16 if hT_out is not None else dt.float32,
                                tag="h_t",
                            )
                            if with_bias:
                                nc.vector.tensor_scalar_mul(
                                    acc[:], acc[:], dinv_sb[:, t : t + 1]
                                )
                                nc.vector.tensor_add(acc[:], acc[:], bias_sb[li][:])
                                if relu:
                                    nc.scalar.activation(h_t[:], acc[:], AF.Relu)
                                else:
                                    nc.scalar.copy(h_t[:], acc[:])
                            else:
                                if relu:
                                    nc.scalar.activation(
                                        h_t[:], acc[:], AF.Relu,
                                        scale=dinv_sb[:, t : t + 1],
                                    )
                                else:
                                    nc.scalar.mul(
                                        h_t[:], acc[:], dinv_sb[:, t : t + 1]
                                    )
                            if hT_out is not None:
                                psT = psT_pool.tile([128, 128], dt.bfloat16, tag="psT")
                                nc.tensor.transpose(psT[:], h_t[:], ident_sb[:])
                                nc.vector.tensor_copy(
                                    hT_out[:, t * 128 : (t + 1) * 128], psT[:]
                                )
                            else:
                                nc.sync.dma_start(
                                    out_p[t * 128 : (t + 1) * 128, :], h_t[:]
                                )

            emit_layers(with_cc=True)
            if reps:
                with tc.For_i(0, reps, 1):
                    emit_layers(with_cc=False)

    nc.compile()
    return nc


_NC_CACHE: dict = {}


def _get_nc(K_lo, K_hi, with_bias):
    key = (K_lo, K_hi, with_bias)
    if key not in _NC_CACHE:
        _NC_CACHE[key] = _build_nc(K_lo, K_hi, with_bias)
    return _NC_CACHE[key]


# ---------------------------------------------------------------------------
# entry point
# ---------------------------------------------------------------------------
def _prepare(x, edge_index, W1, b1, W2, b2, W3, b3):
    import ml_dtypes

    bf16 = ml_dtypes.bfloat16
    x = np.asarray(x, dtype=np.float32)
    W1 = np.asarray(W1, dtype=np.float32)
    W2 = np.asarray(W2, dtype=np.float32)
    W3 = np.asarray(W3, dtype=np.float32)
    b1 = np.asarray(b1, dtype=np.float32)
    b2 = np.asarray(b2, dtype=np.float32)
    b3 = np.asarray(b3, dtype=np.float32)

    plan = _plan(np.asarray(edge_index))
    with_bias = bool(np.any(b1) or np.any(b2) or np.any(b3))
    nc = _get_nc(plan["K_lo"], plan["K_hi"], with_bias)

    iota = np.tile(np.arange(128, dtype=np.float32), (128, 1)).astype(bf16)
    ident = np.eye(128, dtype=np.float32).astype(bf16)
    pos_local = plan["pos_local"]

    in_maps = []
    for k in range(M_CORES):
        pc = plan["per_core"][k]
        v0 = k * NPC
        xT = np.zeros((128, NPAD), dtype=np.float32)
        xT[:, pos_local[v0 : v0 + NPC]] = x[v0 : v0 + NPC].T
        im = dict(
            xT=xT.astype(bf16),
            W1=W1.astype(bf16), W2=W2.astype(bf16), W3=W3.astype(bf16),
            dinv=pc["dinv"], dloc=pc["dloc"], idx=pc["idx"],
            iota=iota, ident=ident,
        )
        if with_bias:
            im["b1r"] = np.tile(b1, (128, 1)).astype(np.float32)
            im["b2r"] = np.tile(b2, (128, 1)).astype(np.float32)
            im["b3r"] = np.tile(b3, (128, 1)).astype(np.float32)
        in_maps.append(im)

    def unpermute(results):
        out = np.empty((N_NODES, OUT_C), dtype=np.float32)
        for k in range(M_CORES):
            v0 = k * NPC
            r = results[k]["out"]
            out[v0 : v0 + NPC] = r[pos_local[v0 : v0 + NPC]]
        return out

    return nc, in_maps, unpermute


def kernel(x, edge_index, W1, b1, W2, b2, W3, b3):
    from concourse.bass_utils import run_bass_kernel_spmd

    nc, in_maps, unpermute = _prepare(x, edge_index, W1, b1, W2, b2, W3, b3)
    res = run_bass_kernel_spmd(nc, in_maps, list(range(M_CORES)))
    return unpermute(res.results)


# revision 27
# speedup vs baseline: 4.9809x; 4.3447x over previous
"""3-layer GCN node classifier on 8 Trainium2 NeuronCores.

Math (per layer, folding the symmetric normalization):
    deg[v]  = in-degree(v) (with self loop), dinv = rsqrt(deg)
    g       = dinv * (h @ W)                  (rows scaled)
    out[c]  = dinv[c] * ( sum_{e: col=c} g[row_e] + g[c] ) + b
    h_next  = relu(out)      (layers 1,2; layer 3 has no relu)

Distribution: nodes are range-sharded across 8 cores (graph parallel).
Each core computes g for its own nodes (dense matmul), the g-table is
all-gathered to every core's HBM in bf16, each core then gathers the
rows for the edges whose *destination* it owns (SWDGE dma_gather over
4 queues, merged across tile blocks) and scatter-adds them into
per-destination-tile PSUM accumulators via one-hot matmuls (bf16) on
the TensorEngine.

Host-side preprocessing only touches edge_index (graph structure):
CSR-style bucketing of edges by destination tile, degree computation,
a within-core node permutation that load-balances destination tiles,
ascending-address sorting of gather indices, and int16 gather-index
packing (the 50k-row table is split into two overlapping <=32768-row
views because SWDGE gather indices are int16).
"""

import math
import os
import numpy as np

# ---------------------------------------------------------------------------
# problem constants (hardcoded per contract; kernel.py must be self-contained)
# ---------------------------------------------------------------------------
N_NODES = 50000
IN_C, HID_C, OUT_C = 128, 128, 64
M_CORES = 8
NPC = N_NODES // M_CORES            # 6250 nodes per core
TPC = (NPC + 127) // 128            # 49 destination tiles per core
NPAD = TPC * 128                    # 6272 padded nodes per core
TBL = M_CORES * NPAD                # 50176 rows in the all-gathered table
LO_CAP = 32768                      # int16 index reach
HI_OFF = max(0, TBL - 32768)        # 17408: hi view = table[HI_OFF:]
NTB = 7                             # destination tiles per gather block
NBLK = TPC // NTB                   # 7 blocks per core

F32 = "float32"


# ---------------------------------------------------------------------------
# host-side graph preprocessing (indices only)
# ---------------------------------------------------------------------------
def _plan(edge_index: np.ndarray):
    """Build per-core index/metadata arrays from edge_index [2, E].

    Chunk structure per destination tile (aligned identity scatter):
      - J_lo aligned-lo chunks: slot d of chunk j holds the j-th lo-view
        edge of destination d (identity scatter matrix, no one-hot);
        missing entries gather a known-zero table row.
      - KO_lo overflow-lo chunks: leftover lo edges, one-hot scattered.
      - J_hi / KO_hi: same for the hi view.
    """
    row = np.asarray(edge_index[0], dtype=np.int64)
    col = np.asarray(edge_index[1], dtype=np.int64)

    deg_in = np.bincount(col, minlength=N_NODES)          # edges only
    dinv = 1.0 / np.sqrt(deg_in + 1.0)                     # + self loop

    # within-core permutation: snake-fill tiles with degree-sorted nodes so
    # every destination tile carries a near-equal number of incoming edges.
    pos_local = np.empty(N_NODES, dtype=np.int64)
    unfilled = []
    for k in range(M_CORES):
        v0 = k * NPC
        d = deg_in[v0 : v0 + NPC]
        order = np.argsort(-d, kind="stable")              # heavy first
        # serpentine tile ids: 0..T-1, T-1..0, ...
        idx = np.arange(NPC)
        rnd, off = divmod(idx, TPC)
        tile_ids = np.where(rnd % 2 == 0, off, TPC - 1 - off)
        slot_in_tile = rnd
        pos = tile_ids * 128 + slot_in_tile
        pos_local[v0 + order] = pos
        used = np.zeros(NPAD, dtype=bool)
        used[pos] = True
        unfilled.append(np.flatnonzero(~used))

    g_pos = (np.arange(N_NODES) // NPC) * NPAD + pos_local  # table row per node
    # known-zero table rows (dinv=0 there -> g rows are exactly 0)
    ZERO_LO = int(unfilled[0][0])                           # core 0 pad row
    ZERO_HI = int(7 * NPAD + unfilled[7][0]) - HI_OFF       # core 7 pad row
    assert 0 <= ZERO_LO < LO_CAP and 0 <= ZERO_HI < 32768

    # per-core per-tile edge buckets
    kd = col // NPC
    src_pos = g_pos[row]
    dst_slot = pos_local[col]
    dst_tile = dst_slot // 128
    dst_loc = dst_slot % 128
    is_lo = src_pos < LO_CAP

    # per-(core,tile,dst) view degrees to size the aligned/overflow split
    key = (kd * TPC + dst_tile) * 128 + dst_loc
    lo_deg = np.bincount(key[is_lo], minlength=M_CORES * TPC * 128).reshape(-1, 128)
    hi_deg = np.bincount(key[~is_lo], minlength=M_CORES * TPC * 128).reshape(-1, 128)
    best = None
    for jl in range(2, 26):
        for jh in range(1, 16):
            ovl = np.maximum(lo_deg - jl, 0).sum(1)
            ovh = np.maximum(hi_deg - jh, 0).sum(1)
            kol = max(int(math.ceil(ovl.max() / 128)), 1)
            koh = max(int(math.ceil(ovh.max() / 128)), 1)
            cost = (jl + jh + kol + koh) + 0.5 * (kol + koh)
            if best is None or cost < best[0]:
                best = (cost, jl, jh, kol, koh)
    _, J_lo, J_hi, KO_lo, KO_hi = best
    K = J_lo + J_hi + KO_lo + KO_hi
    KO = KO_lo + KO_hi
    NCH = TPC * K
    KL = J_lo + KO_lo                                      # lo chunks per tile
    KH = J_hi + KO_hi

    per_core = []
    for k in range(M_CORES):
        # global chunk order per block: tile-major lo runs (J_lo aligned +
        # KO_lo overflow per tile), then tile-major hi runs.
        idx16 = np.zeros((NCH, 128), dtype=np.int16)
        dloc_a = np.full((TPC, KO, 128), 200.0, dtype=np.float32)
        m = kd == k
        tl, lc, sp, lo = dst_tile[m], dst_loc[m], src_pos[m], is_lo[m]
        for b in range(NBLK):
            boff = b * NTB * K
            for tloc in range(NTB):
                t = b * NTB + tloc
                mt = tl == t
                for stream, coff, jn, kon, voff, zrow, ko0 in (
                    (lo & mt, boff + tloc * KL, J_lo, KO_lo, 0, ZERO_LO, 0),
                    ((~lo) & mt, boff + NTB * KL + tloc * KH, J_hi, KO_hi,
                     HI_OFF, ZERO_HI, KO_lo),
                ):
                    sps = sp[stream] - voff
                    lcs = lc[stream]
                    # aligned chunks: slot d of chunk j = j-th edge of dst d
                    al = np.full((jn, 128), zrow, dtype=np.int64)
                    o = np.argsort(lcs, kind="stable")
                    sps, lcs = sps[o], lcs[o]
                    jpos = np.zeros(129, dtype=np.int64)
                    cnt = np.bincount(lcs, minlength=128)
                    within = np.arange(lcs.size) - np.repeat(
                        np.cumsum(cnt) - cnt, cnt
                    )
                    amask = within < jn
                    al[within[amask], lcs[amask]] = sps[amask]
                    idx16[coff : coff + jn] = al.astype(np.int16)
                    # overflow chunks: leftovers, densely packed
                    osp = sps[~amask]
                    olc = lcs[~amask]
                    n = osp.size
                    fi = np.full(kon * 128, zrow, dtype=np.int64)
                    fd = np.full(kon * 128, 200.0, dtype=np.float32)
                    fi[:n] = osp
                    fd[:n] = olc.astype(np.float32)
                    idx16[coff + jn : coff + jn + kon] = (
                        fi.reshape(kon, 128).astype(np.int16)
                    )
                    dloc_a[t, ko0 : ko0 + kon] = fd.reshape(kon, 128)

        # SWDGE wrapped-16 index layout, replicated across the 8 Q7 groups
        flat = idx16.reshape(-1)
        wrapped = flat.reshape(-1, 16).T                    # [16, NCH*8]
        idx_sb = np.tile(wrapped, (8, 1)).copy()            # [128, NCH*8]
        # one-hot table for overflow chunks, tile-major (t, ko)
        oh3d = dloc_a.reshape(TPC * KO, 128)[:, :, None] == np.arange(
            128, dtype=np.float32
        )
        ohtab = (
            oh3d.transpose(1, 0, 2).reshape(128, TPC * KO * 128)
            .astype(np.float32)
        )
        # per-slot dinv (0 on dummy slots)
        dv = np.zeros(NPAD, dtype=np.float32)
        v0 = k * NPC
        dv[pos_local[v0 : v0 + NPC]] = dinv[v0 : v0 + NPC]
        dinv_sb = dv.reshape(TPC, 128).T.copy()             # [128, TPC]
        per_core.append(dict(idx=idx_sb, dinv=dinv_sb, ohtab=ohtab))

    return dict(
        J_lo=J_lo, J_hi=J_hi, KO_lo=KO_lo, KO_hi=KO_hi, NCH=NCH,
        per_core=per_core, pos_local=pos_local, dinv=dinv,
    )


# ---------------------------------------------------------------------------
# device program
# ---------------------------------------------------------------------------
def _build_nc(J_lo: int, J_hi: int, KO_lo: int, KO_hi: int, with_bias: bool,
              reps: int = 0, ablate: str = '',
              nq: int = 4, msg_bufs: int = 3, spk: int = 7):
    abl = set(a for a in ablate.split(',') if a)
    """Build + compile the SPMD program.

    reps > 0 additionally emits a timing loop: the full pipeline runs once
    (correct, fills the gful tables), then a hardware For_i loop re-runs
    the whole body `reps` times with the collectives elided (collectives
    cannot sit inside control flow) so device time dominates wall clock.
    """
    import concourse.bacc as bacc
    import concourse.mybir as mybir
    from concourse import tile
    from concourse._compat import get_trn_type

    dt = mybir.dt
    KL = J_lo + KO_lo
    KH = J_hi + KO_hi
    K = KL + KH
    KO = KO_lo + KO_hi
    NCH = TPC * K
    NW = NCH * 8

    nc = bacc.Bacc(
        get_trn_type() or "TRN2",
        target_bir_lowering=False,
        debug=False,
        enable_asserts=False,
        num_devices=M_CORES,
        num_swdge_queues=nq,
    )

    # I/O (xT/W in bf16; dinv/dloc fp32)
    xT_p = nc.dram_tensor("xT", [128, NPAD], dt.bfloat16, kind="ExternalInput")
    W1_p = nc.dram_tensor("W1", [IN_C, HID_C], dt.bfloat16, kind="ExternalInput")
    W2_p = nc.dram_tensor("W2", [HID_C, HID_C], dt.bfloat16, kind="ExternalInput")
    W3_p = nc.dram_tensor("W3", [HID_C, OUT_C], dt.bfloat16, kind="ExternalInput")
    dinv_p = nc.dram_tensor("dinv", [128, TPC], dt.float32, kind="ExternalInput")
    idx_p = nc.dram_tensor("idx", [128, NW], dt.int16, kind="ExternalInput")
    ident_p = nc.dram_tensor("ident", [128, 128], dt.bfloat16, kind="ExternalInput")
    oh_dt = dt.float8e4
    ohtab_p = nc.dram_tensor("ohtab", [128, TPC * KO * 128], oh_dt, kind="ExternalInput")
    if with_bias:
        b1_p = nc.dram_tensor("b1r", [128, HID_C], dt.float32, kind="ExternalInput")
        b2_p = nc.dram_tensor("b2r", [128, HID_C], dt.float32, kind="ExternalInput")
        b3_p = nc.dram_tensor("b3r", [128, OUT_C], dt.float32, kind="ExternalInput")
    out_p = nc.dram_tensor("out", [NPAD, OUT_C], dt.float32, kind="ExternalOutput")

    RG = [list(range(M_CORES))]
    AF = mybir.ActivationFunctionType
    OP = mybir.AluOpType

    with tile.TileContext(nc) as tc, tc.tile_pool(name="persist", bufs=1) as pp:
        # persistent SBUF tiles (one slot each)
        hT_a = pp.tile([128, NPAD], dt.bfloat16, name="hT_a")
        hT_b = pp.tile([128, NPAD], dt.bfloat16, name="hT_b")
        w1_sb = pp.tile([128, HID_C], dt.bfloat16, name="w1_sb")
        w2_sb = pp.tile([128, HID_C], dt.bfloat16, name="w2_sb")
        w3_sb = pp.tile([128, OUT_C], dt.bfloat16, name="w3_sb")
        dinv_sb = pp.tile([128, TPC], dt.float32, name="dinv_sb")
        idx_sb = pp.tile([128, NW], dt.int16, name="idx_sb")
        ident_sb = pp.tile([128, 128], dt.bfloat16, name="ident_sb")
        bias_sb = []

        nc.sync.dma_start(hT_a[:], xT_p[:])
        nc.sync.dma_start(w1_sb[:], W1_p[:])
        nc.sync.dma_start(w2_sb[:], W2_p[:])
        nc.sync.dma_start(w3_sb[:], W3_p[:])
        nc.sync.dma_start(dinv_sb[:], dinv_p[:])
        nc.sync.dma_start(idx_sb[:], idx_p[:])
        nc.sync.dma_start(ident_sb[:], ident_p[:])
        if with_bias:
            for p, cc in ((b1_p, HID_C), (b2_p, HID_C), (b3_p, OUT_C)):
                t = pp.tile([128, cc], dt.float32, name=f"bias{len(bias_sb)}_sb")
                nc.sync.dma_start(t[:], p[:])
                bias_sb.append(t)

        layers = [
            (w1_sb, HID_C, True, hT_a, hT_b),
            (w2_sb, HID_C, True, hT_b, hT_a),
            (w3_sb, OUT_C, False, hT_a, None),
        ]

        with (
            tc.tile_pool(name="gsb", bufs=2) as gsb_pool,
            tc.tile_pool(name="msg", bufs=msg_bufs) as msg_pool,
            tc.tile_pool(name="oh", bufs=4) as oh_pool,
            tc.tile_pool(name="ohs", bufs=10) as ohs_pool,
            tc.tile_pool(name="eps", bufs=3) as eps_pool,
            tc.tile_pool(name="psA", bufs=2, space="PSUM") as psA_pool,
            tc.tile_pool(name="psS", bufs=2, space="PSUM") as psS_pool,
            tc.tile_pool(name="psT", bufs=2, space="PSUM") as psT_pool,
            tc.tile_pool(name="dram", bufs=1, space="DRAM") as dram_pool,
        ):
            # tables are bf16, layer-3 table padded to 128 cols (upper half
            # junk, never read by compute: matmul rhs slices cols 0:C)
            glocs = [
                dram_pool.tile([NPAD, 128], dt.bfloat16, name=f"gloc{i}")
                for i in range(3)
            ]
            gfuls = [
                dram_pool.tile(
                    [TBL, 128], dt.bfloat16,
                    addr_space="Shared" if M_CORES > 4 else "Local",
                    name=f"gful{i}",
                )
                for i in range(3)
            ]

            def emit_layers(with_cc):
                for li, (w_sb, C, relu, hT_in, hT_out) in enumerate(layers):
                    gloc, gful = glocs[li], gfuls[li]

                    # stage A: g = dinv * (h @ W) for own nodes
                    g_sb = gsb_pool.tile([128, TPC, 128], dt.bfloat16, tag="gsb")
                    for t in range(TPC):
                        psA = psA_pool.tile([128, C], dt.float32, tag="psA")
                        nc.tensor.matmul(
                            psA[:],
                            lhsT=hT_in[:, t * 128 : (t + 1) * 128],
                            rhs=w_sb[:, :C],
                            start=True,
                            stop=True,
                        )
                        nc.vector.tensor_scalar_mul(
                            g_sb[:, t, :C], psA[:], dinv_sb[:, t : t + 1]
                        )
                    nc.sync.dma_start(
                        gloc[:].rearrange("(t p) c -> p t c", p=128), g_sb[:]
                    )

                    # stage B: replicate the g table
                    if with_cc:
                        nc.gpsimd.collective_compute(
                            "AllGather",
                            OP.bypass,
                            replica_groups=RG,
                            ins=[gloc[:]],
                            outs=[gful[:]],
                        )
                    g_lo = gful[0 : min(LO_CAP, TBL), :]
                    g_hi = gful[HI_OFF:TBL, :]

                    # stage C: merged gather per block + one-hot scatter per
                    # destination tile
                    qctr = [0]

                    def gather_run(msg, boff, c0, c1, gview):
                        """One dma_gather over chunk run [c0, c1) of the block."""
                        n = (c1 - c0) * 128
                        nc.gpsimd.dma_gather(
                            msg[:, c0:c1, :],
                            gview[:],
                            idx_sb[:, (boff + c0) * 8 : (boff + c1) * 8],
                            n,
                            n,
                            128,
                            elem_step=128,
                            single_packet=spk > 0,
                            queue_num=qctr[0] % nq,
                        )
                        qctr[0] += 1

                    for b in range(NBLK):
                        boff = b * NTB * K
                        if 'nogather' not in abl:
                            msg = msg_pool.tile(
                                [128, NTB * K, 128], dt.bfloat16, tag="msg"
                            )
                            step = spk if spk > 0 else 10**9
                            for (s0, s1, gv) in (
                                (0, NTB * KL, g_lo),
                                (NTB * KL, NTB * K, g_hi),
                            ):
                                for c0 in range(s0, s1, step):
                                    gather_run(msg, boff, c0, min(c0 + step, s1), gv)
                        for tloc in range(NTB):
                            t = b * NTB + tloc
                            if KO and 'nooh' not in abl:
                                oh_t = ohs_pool.tile([128, KO, 128], oh_dt, tag="ohs")
                                nc.sync.dma_start(
                                    oh_t[:],
                                    ohtab_p[:, t * KO * 128 : (t + 1) * KO * 128]
                                    .rearrange("p (k c) -> p k c", k=KO),
                                )
                            psS = psS_pool.tile([128, C], dt.float32, tag="psS")
                            # (msg chunk within block, overflow one-hot idx or None)
                            chunks = (
                                [(tloc * KL + j, None) for j in range(J_lo)]
                                + [(tloc * KL + J_lo + j, j) for j in range(KO_lo)]
                                + [(NTB * KL + tloc * KH + j, None) for j in range(J_hi)]
                                + [(NTB * KL + tloc * KH + J_hi + j, KO_lo + j)
                                   for j in range(KO_hi)]
                            )
                            if 'nomm' in abl:
                                chunks = chunks[:1]
                            for jj, (mc, ko) in enumerate(chunks):
                                if ko is None or 'nooh' in abl:
                                    mm_lhs = ident_sb[:]
                                else:
                                    mm_lhs = oh_t[:, ko, :]
                                mm_rhs = (
                                    msg[:, mc, :C]
                                    if 'nogather' not in abl
                                    else hT_a[:, jj * 64 : jj * 64 + C]
                                )
                                nc.tensor.matmul(
                                    psS[:],
                                    lhsT=mm_lhs,
                                    rhs=mm_rhs,
                                    start=(jj == 0),
                                    stop=(jj == len(chunks) - 1),
                                )
                            # epilogue: + self-loop, * dinv, (+bias), relu
                            acc = eps_pool.tile([128, C], dt.float32, tag="acc")
                            nc.vector.tensor_add(
                                acc[:], psS[:], g_sb[:, t, :C]
                            )
                            h_t = eps_pool.tile(
                                [128, C],
                                dt.bfloat16 if hT_out is not None else dt.float32,
                                tag="h_t",
                            )
                            if with_bias:
                                nc.vector.tensor_scalar_mul(
                                    acc[:], acc[:], dinv_sb[:, t : t + 1]
                                )
                                nc.vector.tensor_add(acc[:], acc[:], bias_sb[li][:])
                                if relu:
                                    nc.scalar.activation(h_t[:], acc[:], AF.Relu)
                                else:
                                    nc.scalar.copy(h_t[:], acc[:])
                            else:
                                if relu:
                                    nc.scalar.activation(
                                        h_t[:], acc[:], AF.Relu,
                                        scale=dinv_sb[:, t : t + 1],
                                    )
                                else:
                                    nc.scalar.mul(
                                        h_t[:], acc[:], dinv_sb[:, t : t + 1]
                                    )
                            if hT_out is not None:
                                psT = psT_pool.tile([128, 128], dt.bfloat16, tag="psT")
                                nc.tensor.transpose(psT[:], h_t[:], ident_sb[:])
                                nc.vector.tensor_copy(
                                    hT_out[:, t * 128 : (t + 1) * 128], psT[:]
                                )
                            else:
                                nc.sync.dma_start(
                                    out_p[t * 128 : (t + 1) * 128, :], h_t[:]
                                )

            emit_layers(with_cc=True)
            if reps:
                with tc.For_i(0, reps, 1):
                    emit_layers(with_cc=False)

    nc.compile()
    return nc


_NC_CACHE: dict = {}


def _get_nc(J_lo, J_hi, KO_lo, KO_hi, with_bias):
    key = (J_lo, J_hi, KO_lo, KO_hi, with_bias)
    if key not in _NC_CACHE:
        _NC_CACHE[key] = _build_nc(*key)
    return _NC_CACHE[key]


# ---------------------------------------------------------------------------
# entry point
# ---------------------------------------------------------------------------
def _prepare(x, edge_index, W1, b1, W2, b2, W3, b3):
    import ml_dtypes

    bf16 = ml_dtypes.bfloat16
    x = np.asarray(x, dtype=np.float32)
    W1 = np.asarray(W1, dtype=np.float32)
    W2 = np.asarray(W2, dtype=np.float32)
    W3 = np.asarray(W3, dtype=np.float32)
    b1 = np.asarray(b1, dtype=np.float32)
    b2 = np.asarray(b2, dtype=np.float32)
    b3 = np.asarray(b3, dtype=np.float32)

    plan = _plan(np.asarray(edge_index))
    with_bias = bool(np.any(b1) or np.any(b2) or np.any(b3))
    nc = _get_nc(
        plan["J_lo"], plan["J_hi"], plan["KO_lo"], plan["KO_hi"], with_bias
    )

    fp8 = ml_dtypes.float8_e4m3
    ident = np.eye(128, dtype=np.float32).astype(bf16)
    pos_local = plan["pos_local"]

    in_maps = []
    for k in range(M_CORES):
        pc = plan["per_core"][k]
        v0 = k * NPC
        xT = np.zeros((128, NPAD), dtype=np.float32)
        xT[:, pos_local[v0 : v0 + NPC]] = x[v0 : v0 + NPC].T
        im = dict(
            xT=xT.astype(bf16),
            W1=W1.astype(bf16), W2=W2.astype(bf16), W3=W3.astype(bf16),
            dinv=pc["dinv"], idx=pc["idx"],
            ident=ident, ohtab=pc["ohtab"].astype(fp8),
        )
        if with_bias:
            im["b1r"] = np.tile(b1, (128, 1)).astype(np.float32)
            im["b2r"] = np.tile(b2, (128, 1)).astype(np.float32)
            im["b3r"] = np.tile(b3, (128, 1)).astype(np.float32)
        in_maps.append(im)

    def unpermute(results):
        out = np.empty((N_NODES, OUT_C), dtype=np.float32)
        for k in range(M_CORES):
            v0 = k * NPC
            r = results[k]["out"]
            out[v0 : v0 + NPC] = r[pos_local[v0 : v0 + NPC]]
        return out

    return nc, in_maps, unpermute


def kernel(x, edge_index, W1, b1, W2, b2, W3, b3):
    from concourse.bass_utils import run_bass_kernel_spmd

    nc, in_maps, unpermute = _prepare(x, edge_index, W1, b1, W2, b2, W3, b3)
    res = run_bass_kernel_spmd(nc, in_maps, list(range(M_CORES)))
    return unpermute(res.results)


# revision 28
# speedup vs baseline: 5.3250x; 1.0691x over previous
"""3-layer GCN node classifier on 8 Trainium2 NeuronCores.

Math (per layer, folding the symmetric normalization):
    deg[v]  = in-degree(v) (with self loop), dinv = rsqrt(deg)
    g       = dinv * (h @ W)                  (rows scaled)
    out[c]  = dinv[c] * ( sum_{e: col=c} g[row_e] + g[c] ) + b
    h_next  = relu(out)      (layers 1,2; layer 3 has no relu)

Distribution: nodes are range-sharded across 8 cores (graph parallel).
Each core computes g for its own nodes (dense matmul), the g-table is
all-gathered to every core's HBM in bf16, each core then gathers the
rows for the edges whose *destination* it owns (SWDGE dma_gather over
4 queues, merged across tile blocks) and scatter-adds them into
per-destination-tile PSUM accumulators via one-hot matmuls (bf16) on
the TensorEngine.

Host-side preprocessing only touches edge_index (graph structure):
CSR-style bucketing of edges by destination tile, degree computation,
a within-core node permutation that load-balances destination tiles,
ascending-address sorting of gather indices, and int16 gather-index
packing (the 50k-row table is split into two overlapping <=32768-row
views because SWDGE gather indices are int16).
"""

import math
import os
import numpy as np

# ---------------------------------------------------------------------------
# problem constants (hardcoded per contract; kernel.py must be self-contained)
# ---------------------------------------------------------------------------
N_NODES = 50000
IN_C, HID_C, OUT_C = 128, 128, 64
M_CORES = 8
NPC = N_NODES // M_CORES            # 6250 nodes per core
TPC = (NPC + 127) // 128            # 49 destination tiles per core
NPAD = TPC * 128                    # 6272 padded nodes per core
TBL = M_CORES * NPAD                # 50176 rows in the all-gathered table
LO_CAP = 32768                      # int16 index reach
HI_OFF = max(0, TBL - 32768)        # 17408: hi view = table[HI_OFF:]
NTB = 7                             # destination tiles per gather block
NBLK = TPC // NTB                   # 7 blocks per core

F32 = "float32"


# ---------------------------------------------------------------------------
# host-side graph preprocessing (indices only)
# ---------------------------------------------------------------------------
def _plan(edge_index: np.ndarray):
    """Build per-core index/metadata arrays from edge_index [2, E].

    Chunk structure per destination tile (aligned identity scatter):
      - J_lo aligned-lo chunks: slot d of chunk j holds the j-th lo-view
        edge of destination d (identity scatter matrix, no one-hot);
        missing entries gather a known-zero table row.
      - KO_lo overflow-lo chunks: leftover lo edges, one-hot scattered.
      - J_hi / KO_hi: same for the hi view.
    """
    row = np.asarray(edge_index[0], dtype=np.int64)
    col = np.asarray(edge_index[1], dtype=np.int64)

    deg_in = np.bincount(col, minlength=N_NODES)          # edges only
    dinv = 1.0 / np.sqrt(deg_in + 1.0)                     # + self loop

    # within-core permutation: snake-fill tiles with degree-sorted nodes so
    # every destination tile carries a near-equal number of incoming edges.
    pos_local = np.empty(N_NODES, dtype=np.int64)
    unfilled = []
    for k in range(M_CORES):
        v0 = k * NPC
        d = deg_in[v0 : v0 + NPC]
        order = np.argsort(-d, kind="stable")              # heavy first
        # serpentine tile ids: 0..T-1, T-1..0, ...
        idx = np.arange(NPC)
        rnd, off = divmod(idx, TPC)
        tile_ids = np.where(rnd % 2 == 0, off, TPC - 1 - off)
        slot_in_tile = rnd
        pos = tile_ids * 128 + slot_in_tile
        pos_local[v0 + order] = pos
        used = np.zeros(NPAD, dtype=bool)
        used[pos] = True
        unfilled.append(np.flatnonzero(~used))

    g_pos = (np.arange(N_NODES) // NPC) * NPAD + pos_local  # table row per node
    # known-zero table rows (dinv=0 there -> g rows are exactly 0)
    ZERO_LO = int(unfilled[0][0])                           # core 0 pad row
    ZERO_HI = int(7 * NPAD + unfilled[7][0]) - HI_OFF       # core 7 pad row
    assert 0 <= ZERO_LO < LO_CAP and 0 <= ZERO_HI < 32768

    # per-core per-tile edge buckets
    kd = col // NPC
    src_pos = g_pos[row]
    dst_slot = pos_local[col]
    dst_tile = dst_slot // 128
    dst_loc = dst_slot % 128
    is_lo = src_pos < LO_CAP

    # per-(core,tile,dst) view degrees to size the aligned/overflow split
    key = (kd * TPC + dst_tile) * 128 + dst_loc
    lo_deg = np.bincount(key[is_lo], minlength=M_CORES * TPC * 128).reshape(-1, 128)
    hi_deg = np.bincount(key[~is_lo], minlength=M_CORES * TPC * 128).reshape(-1, 128)
    best = None
    for jl in range(2, 26):
        for jh in range(1, 16):
            ovl = np.maximum(lo_deg - jl, 0).sum(1)
            ovh = np.maximum(hi_deg - jh, 0).sum(1)
            kol = max(int(math.ceil(ovl.max() / 128)), 1)
            koh = max(int(math.ceil(ovh.max() / 128)), 1)
            cost = (jl + jh + kol + koh) + 0.3 * (kol + koh)
            if best is None or cost < best[0]:
                best = (cost, jl, jh, kol, koh)
    _, J_lo, J_hi, KO_lo, KO_hi = best
    K = J_lo + J_hi + KO_lo + KO_hi
    KO = KO_lo + KO_hi
    NCH = TPC * K
    KL = J_lo + KO_lo                                      # lo chunks per tile
    KH = J_hi + KO_hi

    per_core = []
    for k in range(M_CORES):
        # global chunk order per block: tile-major lo runs (J_lo aligned +
        # KO_lo overflow per tile), then tile-major hi runs.
        idx16 = np.zeros((NCH, 128), dtype=np.int16)
        dloc_a = np.full((TPC, KO, 128), 200.0, dtype=np.float32)
        m = kd == k
        tl, lc, sp, lo = dst_tile[m], dst_loc[m], src_pos[m], is_lo[m]
        for b in range(NBLK):
            boff = b * NTB * K
            for tloc in range(NTB):
                t = b * NTB + tloc
                mt = tl == t
                for stream, coff, jn, kon, voff, zrow, ko0 in (
                    (lo & mt, boff + tloc * KL, J_lo, KO_lo, 0, ZERO_LO, 0),
                    ((~lo) & mt, boff + NTB * KL + tloc * KH, J_hi, KO_hi,
                     HI_OFF, ZERO_HI, KO_lo),
                ):
                    sps = sp[stream] - voff
                    lcs = lc[stream]
                    # aligned chunks: slot d of chunk j = j-th edge of dst d
                    al = np.full((jn, 128), zrow, dtype=np.int64)
                    o = np.argsort(lcs, kind="stable")
                    sps, lcs = sps[o], lcs[o]
                    jpos = np.zeros(129, dtype=np.int64)
                    cnt = np.bincount(lcs, minlength=128)
                    within = np.arange(lcs.size) - np.repeat(
                        np.cumsum(cnt) - cnt, cnt
                    )
                    amask = within < jn
                    al[within[amask], lcs[amask]] = sps[amask]
                    idx16[coff : coff + jn] = al.astype(np.int16)
                    # overflow chunks: leftovers, densely packed
                    osp = sps[~amask]
                    olc = lcs[~amask]
                    n = osp.size
                    fi = np.full(kon * 128, zrow, dtype=np.int64)
                    fd = np.full(kon * 128, 200.0, dtype=np.float32)
                    fi[:n] = osp
                    fd[:n] = olc.astype(np.float32)
                    idx16[coff + jn : coff + jn + kon] = (
                        fi.reshape(kon, 128).astype(np.int16)
                    )
                    dloc_a[t, ko0 : ko0 + kon] = fd.reshape(kon, 128)

        # SWDGE wrapped-16 index layout, replicated across the 8 Q7 groups
        flat = idx16.reshape(-1)
        wrapped = flat.reshape(-1, 16).T                    # [16, NCH*8]
        idx_sb = np.tile(wrapped, (8, 1)).copy()            # [128, NCH*8]
        # one-hot table for overflow chunks, tile-major (t, ko)
        oh3d = dloc_a.reshape(TPC * KO, 128)[:, :, None] == np.arange(
            128, dtype=np.float32
        )
        ohtab = (
            oh3d.transpose(1, 0, 2).reshape(128, TPC * KO * 128)
            .astype(np.float32)
        )
        # per-slot dinv (0 on dummy slots)
        dv = np.zeros(NPAD, dtype=np.float32)
        v0 = k * NPC
        dv[pos_local[v0 : v0 + NPC]] = dinv[v0 : v0 + NPC]
        dinv_sb = dv.reshape(TPC, 128).T.copy()             # [128, TPC]
        per_core.append(dict(idx=idx_sb, dinv=dinv_sb, ohtab=ohtab))

    return dict(
        J_lo=J_lo, J_hi=J_hi, KO_lo=KO_lo, KO_hi=KO_hi, NCH=NCH,
        per_core=per_core, pos_local=pos_local, dinv=dinv,
    )


# ---------------------------------------------------------------------------
# device program
# ---------------------------------------------------------------------------
def _build_nc(J_lo: int, J_hi: int, KO_lo: int, KO_hi: int, with_bias: bool,
              reps: int = 0, ablate: str = '',
              nq: int = 4, msg_bufs: int = 3, spk: int = 7):
    abl = set(a for a in ablate.split(',') if a)
    """Build + compile the SPMD program.

    reps > 0 additionally emits a timing loop: the full pipeline runs once
    (correct, fills the gful tables), then a hardware For_i loop re-runs
    the whole body `reps` times with the collectives elided (collectives
    cannot sit inside control flow) so device time dominates wall clock.
    """
    import concourse.bacc as bacc
    import concourse.mybir as mybir
    from concourse import tile
    from concourse._compat import get_trn_type

    dt = mybir.dt
    KL = J_lo + KO_lo
    KH = J_hi + KO_hi
    K = KL + KH
    KO = KO_lo + KO_hi
    NCH = TPC * K
    NW = NCH * 8

    nc = bacc.Bacc(
        get_trn_type() or "TRN2",
        target_bir_lowering=False,
        debug=False,
        enable_asserts=False,
        num_devices=M_CORES,
        num_swdge_queues=nq,
    )

    # I/O (xT/W in bf16; dinv/dloc fp32)
    xT_p = nc.dram_tensor("xT", [128, NPAD], dt.bfloat16, kind="ExternalInput")
    W1_p = nc.dram_tensor("W1", [IN_C, HID_C], dt.bfloat16, kind="ExternalInput")
    W2_p = nc.dram_tensor("W2", [HID_C, HID_C], dt.bfloat16, kind="ExternalInput")
    W3_p = nc.dram_tensor("W3", [HID_C, OUT_C], dt.bfloat16, kind="ExternalInput")
    dinv_p = nc.dram_tensor("dinv", [128, TPC], dt.float32, kind="ExternalInput")
    idx_p = nc.dram_tensor("idx", [128, NW], dt.int16, kind="ExternalInput")
    ident_p = nc.dram_tensor("ident", [128, 128], dt.bfloat16, kind="ExternalInput")
    oh_dt = dt.float8e4
    ohtab_p = nc.dram_tensor("ohtab", [128, TPC * KO * 128], oh_dt, kind="ExternalInput")
    if with_bias:
        b1_p = nc.dram_tensor("b1r", [128, HID_C], dt.float32, kind="ExternalInput")
        b2_p = nc.dram_tensor("b2r", [128, HID_C], dt.float32, kind="ExternalInput")
        b3_p = nc.dram_tensor("b3r", [128, OUT_C], dt.float32, kind="ExternalInput")
    out_p = nc.dram_tensor("out", [NPAD, OUT_C], dt.float32, kind="ExternalOutput")

    RG = [list(range(M_CORES))]
    AF = mybir.ActivationFunctionType
    OP = mybir.AluOpType

    with tile.TileContext(nc) as tc, tc.tile_pool(name="persist", bufs=1) as pp:
        # persistent SBUF tiles (one slot each)
        hT_a = pp.tile([128, NPAD], dt.bfloat16, name="hT_a")
        hT_b = pp.tile([128, NPAD], dt.bfloat16, name="hT_b")
        w1_sb = pp.tile([128, HID_C], dt.bfloat16, name="w1_sb")
        w2_sb = pp.tile([128, HID_C], dt.bfloat16, name="w2_sb")
        w3_sb = pp.tile([128, OUT_C], dt.bfloat16, name="w3_sb")
        dinv_sb = pp.tile([128, TPC], dt.float32, name="dinv_sb")
        idx_sb = pp.tile([128, NW], dt.int16, name="idx_sb")
        ident_sb = pp.tile([128, 128], dt.bfloat16, name="ident_sb")
        bias_sb = []

        nc.sync.dma_start(hT_a[:], xT_p[:])
        nc.sync.dma_start(w1_sb[:], W1_p[:])
        nc.sync.dma_start(w2_sb[:], W2_p[:])
        nc.sync.dma_start(w3_sb[:], W3_p[:])
        nc.sync.dma_start(dinv_sb[:], dinv_p[:])
        nc.sync.dma_start(idx_sb[:], idx_p[:])
        nc.sync.dma_start(ident_sb[:], ident_p[:])
        if with_bias:
            for p, cc in ((b1_p, HID_C), (b2_p, HID_C), (b3_p, OUT_C)):
                t = pp.tile([128, cc], dt.float32, name=f"bias{len(bias_sb)}_sb")
                nc.sync.dma_start(t[:], p[:])
                bias_sb.append(t)

        layers = [
            (w1_sb, HID_C, True, hT_a, hT_b),
            (w2_sb, HID_C, True, hT_b, hT_a),
            (w3_sb, OUT_C, False, hT_a, None),
        ]

        with (
            tc.tile_pool(name="gsb", bufs=2) as gsb_pool,
            tc.tile_pool(name="msg", bufs=msg_bufs) as msg_pool,
            tc.tile_pool(name="oh", bufs=4) as oh_pool,
            tc.tile_pool(name="ohs", bufs=10) as ohs_pool,
            tc.tile_pool(name="eps", bufs=3) as eps_pool,
            tc.tile_pool(name="psA", bufs=2, space="PSUM") as psA_pool,
            tc.tile_pool(name="psS", bufs=2, space="PSUM") as psS_pool,
            tc.tile_pool(name="psT", bufs=2, space="PSUM") as psT_pool,
            tc.tile_pool(name="dram", bufs=1, space="DRAM") as dram_pool,
        ):
            # tables are bf16, layer-3 table padded to 128 cols (upper half
            # junk, never read by compute: matmul rhs slices cols 0:C)
            glocs = [
                dram_pool.tile([NPAD, 128], dt.bfloat16, name=f"gloc{i}")
                for i in range(3)
            ]
            gfuls = [
                dram_pool.tile(
                    [TBL, 128], dt.bfloat16,
                    addr_space="Shared" if M_CORES > 4 else "Local",
                    name=f"gful{i}",
                )
                for i in range(3)
            ]

            def emit_layers(with_cc):
                for li, (w_sb, C, relu, hT_in, hT_out) in enumerate(layers):
                    gloc, gful = glocs[li], gfuls[li]

                    # stage A: g = dinv * (h @ W) for own nodes
                    g_sb = gsb_pool.tile([128, TPC, 128], dt.bfloat16, tag="gsb")
                    for t in range(TPC):
                        psA = psA_pool.tile([128, C], dt.float32, tag="psA")
                        nc.tensor.matmul(
                            psA[:],
                            lhsT=hT_in[:, t * 128 : (t + 1) * 128],
                            rhs=w_sb[:, :C],
                            start=True,
                            stop=True,
                        )
                        nc.vector.tensor_scalar_mul(
                            g_sb[:, t, :C], psA[:], dinv_sb[:, t : t + 1]
                        )
                    nc.sync.dma_start(
                        gloc[:].rearrange("(t p) c -> p t c", p=128), g_sb[:]
                    )

                    # stage B: replicate the g table
                    if with_cc:
                        nc.gpsimd.collective_compute(
                            "AllGather",
                            OP.bypass,
                            replica_groups=RG,
                            ins=[gloc[:]],
                            outs=[gful[:]],
                        )
                    g_lo = gful[0 : min(LO_CAP, TBL), :]
                    g_hi = gful[HI_OFF:TBL, :]

                    # stage C: merged gather per block + one-hot scatter per
                    # destination tile
                    qctr = [0]

                    def gather_run(msg, boff, c0, c1, gview):
                        """One dma_gather over chunk run [c0, c1) of the block."""
                        n = (c1 - c0) * 128
                        nc.gpsimd.dma_gather(
                            msg[:, c0:c1, :],
                            gview[:],
                            idx_sb[:, (boff + c0) * 8 : (boff + c1) * 8],
                            n,
                            n,
                            128,
                            elem_step=128,
                            single_packet=spk > 0,
                            queue_num=qctr[0] % nq,
                        )
                        qctr[0] += 1

                    for b in range(NBLK):
                        boff = b * NTB * K
                        if 'nogather' not in abl:
                            msg = msg_pool.tile(
                                [128, NTB * K, 128], dt.bfloat16, tag="msg"
                            )
                            step = spk if spk > 0 else 10**9
                            for (s0, s1, gv) in (
                                (0, NTB * KL, g_lo),
                                (NTB * KL, NTB * K, g_hi),
                            ):
                                for c0 in range(s0, s1, step):
                                    gather_run(msg, boff, c0, min(c0 + step, s1), gv)
                        for tloc in range(NTB):
                            t = b * NTB + tloc
                            if KO and 'nooh' not in abl:
                                oh_t = ohs_pool.tile([128, KO, 128], oh_dt, tag="ohs")
                                nc.sync.dma_start(
                                    oh_t[:],
                                    ohtab_p[:, t * KO * 128 : (t + 1) * KO * 128]
                                    .rearrange("p (k c) -> p k c", k=KO),
                                )
                            psS = psS_pool.tile([128, C], dt.float32, tag="psS")
                            # (msg chunk within block, overflow one-hot idx or None)
                            chunks = (
                                [(tloc * KL + j, None) for j in range(J_lo)]
                                + [(tloc * KL + J_lo + j, j) for j in range(KO_lo)]
                                + [(NTB * KL + tloc * KH + j, None) for j in range(J_hi)]
                                + [(NTB * KL + tloc * KH + J_hi + j, KO_lo + j)
                                   for j in range(KO_hi)]
                            )
                            if 'nomm' in abl:
                                chunks = chunks[:1]
                            for jj, (mc, ko) in enumerate(chunks):
                                if ko is None or 'nooh' in abl:
                                    mm_lhs = ident_sb[:]
                                else:
                                    mm_lhs = oh_t[:, ko, :]
                                mm_rhs = (
                                    msg[:, mc, :C]
                                    if 'nogather' not in abl
                                    else hT_a[:, jj * 64 : jj * 64 + C]
                                )
                                nc.tensor.matmul(
                                    psS[:],
                                    lhsT=mm_lhs,
                                    rhs=mm_rhs,
                                    start=(jj == 0),
                                    stop=(jj == len(chunks) - 1),
                                )
                            # epilogue: + self-loop, * dinv, (+bias), relu
                            acc = eps_pool.tile([128, C], dt.float32, tag="acc")
                            nc.vector.tensor_add(
                                acc[:], psS[:], g_sb[:, t, :C]
                            )
                            h_t = eps_pool.tile(
                                [128, C],
                                dt.bfloat16 if hT_out is not None else dt.float32,
                                tag="h_t",
                            )
                            if with_bias:
                                nc.vector.tensor_scalar_mul(
                                    acc[:], acc[:], dinv_sb[:, t : t + 1]
                                )
                                nc.vector.tensor_add(acc[:], acc[:], bias_sb[li][:])
                                if relu:
                                    nc.scalar.activation(h_t[:], acc[:], AF.Relu)
                                else:
                                    nc.scalar.copy(h_t[:], acc[:])
                            else:
                                if relu:
                                    nc.scalar.activation(
                                        h_t[:], acc[:], AF.Relu,
                                        scale=dinv_sb[:, t : t + 1],
                                    )
                                else:
                                    nc.scalar.mul(
                                        h_t[:], acc[:], dinv_sb[:, t : t + 1]
                                    )
                            if hT_out is not None:
                                psT = psT_pool.tile([128, 128], dt.bfloat16, tag="psT")
                                nc.tensor.transpose(psT[:], h_t[:], ident_sb[:])
                                nc.vector.tensor_copy(
                                    hT_out[:, t * 128 : (t + 1) * 128], psT[:]
                                )
                            else:
                                nc.sync.dma_start(
                                    out_p[t * 128 : (t + 1) * 128, :], h_t[:]
                                )

            emit_layers(with_cc=True)
            if reps:
                with tc.For_i(0, reps, 1):
                    emit_layers(with_cc=False)

    nc.compile()
    return nc


_NC_CACHE: dict = {}


def _get_nc(J_lo, J_hi, KO_lo, KO_hi, with_bias):
    key = (J_lo, J_hi, KO_lo, KO_hi, with_bias)
    if key not in _NC_CACHE:
        _NC_CACHE[key] = _build_nc(*key)
    return _NC_CACHE[key]


# ---------------------------------------------------------------------------
# entry point
# ---------------------------------------------------------------------------
def _prepare(x, edge_index, W1, b1, W2, b2, W3, b3):
    import ml_dtypes

    bf16 = ml_dtypes.bfloat16
    x = np.asarray(x, dtype=np.float32)
    W1 = np.asarray(W1, dtype=np.float32)
    W2 = np.asarray(W2, dtype=np.float32)
    W3 = np.asarray(W3, dtype=np.float32)
    b1 = np.asarray(b1, dtype=np.float32)
    b2 = np.asarray(b2, dtype=np.float32)
    b3 = np.asarray(b3, dtype=np.float32)

    plan = _plan(np.asarray(edge_index))
    with_bias = bool(np.any(b1) or np.any(b2) or np.any(b3))
    nc = _get_nc(
        plan["J_lo"], plan["J_hi"], plan["KO_lo"], plan["KO_hi"], with_bias
    )

    fp8 = ml_dtypes.float8_e4m3
    ident = np.eye(128, dtype=np.float32).astype(bf16)
    pos_local = plan["pos_local"]

    in_maps = []
    for k in range(M_CORES):
        pc = plan["per_core"][k]
        v0 = k * NPC
        xT = np.zeros((128, NPAD), dtype=np.float32)
        xT[:, pos_local[v0 : v0 + NPC]] = x[v0 : v0 + NPC].T
        im = dict(
            xT=xT.astype(bf16),
            W1=W1.astype(bf16), W2=W2.astype(bf16), W3=W3.astype(bf16),
            dinv=pc["dinv"], idx=pc["idx"],
            ident=ident, ohtab=pc["ohtab"].astype(fp8),
        )
        if with_bias:
            im["b1r"] = np.tile(b1, (128, 1)).astype(np.float32)
            im["b2r"] = np.tile(b2, (128, 1)).astype(np.float32)
            im["b3r"] = np.tile(b3, (128, 1)).astype(np.float32)
        in_maps.append(im)

    def unpermute(results):
        out = np.empty((N_NODES, OUT_C), dtype=np.float32)
        for k in range(M_CORES):
            v0 = k * NPC
            r = results[k]["out"]
            out[v0 : v0 + NPC] = r[pos_local[v0 : v0 + NPC]]
        return out

    return nc, in_maps, unpermute


def kernel(x, edge_index, W1, b1, W2, b2, W3, b3):
    from concourse.bass_utils import run_bass_kernel_spmd

    nc, in_maps, unpermute = _prepare(x, edge_index, W1, b1, W2, b2, W3, b3)
    res = run_bass_kernel_spmd(nc, in_maps, list(range(M_CORES)))
    return unpermute(res.results)


# revision 33
# speedup vs baseline: 14.1784x; 2.6626x over previous
"""3-layer GCN node classifier on 8 Trainium2 NeuronCores.

Math (per layer, folding the symmetric normalization):
    deg[v]  = in-degree(v) (with self loop), dinv = rsqrt(deg)
    g       = dinv * (h @ W)                  (rows scaled)
    out[c]  = dinv[c] * ( sum_{e: col=c} g[row_e] + g[c] ) + b
    h_next  = relu(out)      (layers 1,2; layer 3 has no relu)

Distribution: nodes are range-sharded across 8 cores (graph parallel).
Each core computes g for its own nodes (dense matmul), the g-table is
all-gathered to every core's HBM in bf16, each core then gathers the
rows for the edges whose *destination* it owns (SWDGE dma_gather over
4 queues, merged across tile blocks) and scatter-adds them into
per-destination-tile PSUM accumulators via one-hot matmuls (bf16) on
the TensorEngine.

Host-side preprocessing only touches edge_index (graph structure):
CSR-style bucketing of edges by destination tile, degree computation,
a within-core node permutation that load-balances destination tiles,
ascending-address sorting of gather indices, and int16 gather-index
packing (the 50k-row table is split into two overlapping <=32768-row
views because SWDGE gather indices are int16).
"""

import math
import os
import numpy as np

# ---------------------------------------------------------------------------
# problem constants (hardcoded per contract; kernel.py must be self-contained)
# ---------------------------------------------------------------------------
N_NODES = 50000
IN_C, HID_C, OUT_C = 128, 128, 64
M_CORES = 8
NPC = N_NODES // M_CORES            # 6250 nodes per core
TPC = (NPC + 127) // 128            # 49 destination tiles per core
NPAD = TPC * 128                    # 6272 padded nodes per core
TBL = M_CORES * NPAD                # 50176 rows in the all-gathered table
LO_CAP = 32768                      # int16 index reach
HI_OFF = max(0, TBL - 32768)        # 17408: hi view = table[HI_OFF:]
NTB = 7                             # destination tiles per gather block
NBLK = TPC // NTB                   # 7 blocks per core

F32 = "float32"


# ---------------------------------------------------------------------------
# host-side graph preprocessing (indices only)
# ---------------------------------------------------------------------------
def _plan(edge_index: np.ndarray):
    """Build per-core index/metadata arrays from edge_index [2, E].

    Chunk structure per destination tile (aligned identity scatter):
      - J_lo aligned-lo chunks: slot d of chunk j holds the j-th lo-view
        edge of destination d (identity scatter matrix, no one-hot);
        missing entries gather a known-zero table row.
      - KO_lo overflow-lo chunks: leftover lo edges, one-hot scattered.
      - J_hi / KO_hi: same for the hi view.
    """
    row = np.asarray(edge_index[0], dtype=np.int64)
    col = np.asarray(edge_index[1], dtype=np.int64)

    deg_in = np.bincount(col, minlength=N_NODES)          # edges only
    dinv = 1.0 / np.sqrt(deg_in + 1.0)                     # + self loop

    # within-core permutation: snake-fill tiles with degree-sorted nodes so
    # every destination tile carries a near-equal number of incoming edges.
    pos_local = np.empty(N_NODES, dtype=np.int64)
    unfilled = []
    for k in range(M_CORES):
        v0 = k * NPC
        d = deg_in[v0 : v0 + NPC]
        order = np.argsort(-d, kind="stable")              # heavy first
        # serpentine tile ids: 0..T-1, T-1..0, ...
        idx = np.arange(NPC)
        rnd, off = divmod(idx, TPC)
        tile_ids = np.where(rnd % 2 == 0, off, TPC - 1 - off)
        slot_in_tile = rnd
        pos = tile_ids * 128 + slot_in_tile
        pos_local[v0 + order] = pos
        used = np.zeros(NPAD, dtype=bool)
        used[pos] = True
        unfilled.append(np.flatnonzero(~used))

    g_pos = (np.arange(N_NODES) // NPC) * NPAD + pos_local  # table row per node
    # known-zero table rows (dinv=0 there -> g rows are exactly 0)
    ZERO_LO = int(unfilled[0][0])                           # core 0 pad row
    ZERO_HI = int(7 * NPAD + unfilled[7][0]) - HI_OFF       # core 7 pad row
    assert 0 <= ZERO_LO < LO_CAP and 0 <= ZERO_HI < 32768

    # per-core per-tile edge buckets
    kd = col // NPC
    src_pos = g_pos[row]
    dst_slot = pos_local[col]
    dst_tile = dst_slot // 128
    dst_loc = dst_slot % 128
    is_lo = src_pos < LO_CAP

    # per-(core,tile,dst) view degrees to size the aligned/overflow split
    key = (kd * TPC + dst_tile) * 128 + dst_loc
    lo_deg = np.bincount(key[is_lo], minlength=M_CORES * TPC * 128).reshape(-1, 128)
    hi_deg = np.bincount(key[~is_lo], minlength=M_CORES * TPC * 128).reshape(-1, 128)
    best = None
    for jl in range(2, 26):
        for jh in range(1, 16):
            ovl = np.maximum(lo_deg - jl, 0).sum(1)
            ovh = np.maximum(hi_deg - jh, 0).sum(1)
            kol = max(int(math.ceil(ovl.max() / 128)), 1)
            koh = max(int(math.ceil(ovh.max() / 128)), 1)
            cost = (jl + jh + kol + koh) + 0.3 * (kol + koh)
            if best is None or cost < best[0]:
                best = (cost, jl, jh, kol, koh)
    _, J_lo, J_hi, KO_lo, KO_hi = best
    K = J_lo + J_hi + KO_lo + KO_hi
    KO = KO_lo + KO_hi
    NCH = TPC * K
    KL = J_lo + KO_lo                                      # lo chunks per tile
    KH = J_hi + KO_hi

    per_core = []
    for k in range(M_CORES):
        # global chunk order per block: tile-major lo runs (J_lo aligned +
        # KO_lo overflow per tile), then tile-major hi runs.
        idx16 = np.zeros((NCH, 128), dtype=np.int16)
        dloc_a = np.full((TPC, KO, 128), 200.0, dtype=np.float32)
        m = kd == k
        tl, lc, sp, lo = dst_tile[m], dst_loc[m], src_pos[m], is_lo[m]
        for b in range(NBLK):
            boff = b * NTB * K
            for tloc in range(NTB):
                t = b * NTB + tloc
                mt = tl == t
                for stream, coff, jn, kon, voff, zrow, ko0 in (
                    (lo & mt, boff + tloc * KL, J_lo, KO_lo, 0, ZERO_LO, 0),
                    ((~lo) & mt, boff + NTB * KL + tloc * KH, J_hi, KO_hi,
                     HI_OFF, ZERO_HI, KO_lo),
                ):
                    sps = sp[stream] - voff
                    lcs = lc[stream]
                    # aligned chunks: slot d of chunk j = j-th edge of dst d
                    al = np.full((jn, 128), zrow, dtype=np.int64)
                    o = np.argsort(lcs, kind="stable")
                    sps, lcs = sps[o], lcs[o]
                    jpos = np.zeros(129, dtype=np.int64)
                    cnt = np.bincount(lcs, minlength=128)
                    within = np.arange(lcs.size) - np.repeat(
                        np.cumsum(cnt) - cnt, cnt
                    )
                    amask = within < jn
                    al[within[amask], lcs[amask]] = sps[amask]
                    idx16[coff : coff + jn] = al.astype(np.int16)
                    # overflow chunks: leftovers, densely packed
                    osp = sps[~amask]
                    olc = lcs[~amask]
                    n = osp.size
                    fi = np.full(kon * 128, zrow, dtype=np.int64)
                    fd = np.full(kon * 128, 200.0, dtype=np.float32)
                    fi[:n] = osp
                    fd[:n] = olc.astype(np.float32)
                    idx16[coff + jn : coff + jn + kon] = (
                        fi.reshape(kon, 128).astype(np.int16)
                    )
                    dloc_a[t, ko0 : ko0 + kon] = fd.reshape(kon, 128)

        # SWDGE wrapped-16 index layout, replicated across the 8 Q7 groups
        flat = idx16.reshape(-1)
        wrapped = flat.reshape(-1, 16).T                    # [16, NCH*8]
        idx_sb = np.tile(wrapped, (8, 1)).copy()            # [128, NCH*8]
        # one-hot table for overflow chunks, tile-major (t, ko)
        oh3d = dloc_a.reshape(TPC * KO, 128)[:, :, None] == np.arange(
            128, dtype=np.float32
        )
        ohtab = (
            oh3d.transpose(1, 0, 2).reshape(128, TPC * KO * 128)
            .astype(np.float32)
        )
        # per-slot dinv (0 on dummy slots)
        dv = np.zeros(NPAD, dtype=np.float32)
        v0 = k * NPC
        dv[pos_local[v0 : v0 + NPC]] = dinv[v0 : v0 + NPC]
        dinv_sb = dv.reshape(TPC, 128).T.copy()             # [128, TPC]
        per_core.append(dict(idx=idx_sb, dinv=dinv_sb, ohtab=ohtab))

    return dict(
        J_lo=J_lo, J_hi=J_hi, KO_lo=KO_lo, KO_hi=KO_hi, NCH=NCH,
        per_core=per_core, pos_local=pos_local, dinv=dinv,
    )


# ---------------------------------------------------------------------------
# device program
# ---------------------------------------------------------------------------
def _build_nc(J_lo: int, J_hi: int, KO_lo: int, KO_hi: int, with_bias: bool,
              reps: int = 0, ablate: str = '',
              nq: int = 4, msg_bufs: int = 3, spk: int = 7):
    abl = set(a for a in ablate.split(',') if a)
    """Build + compile the SPMD program.

    reps > 0 additionally emits a timing loop: the full pipeline runs once
    (correct, fills the gful tables), then a hardware For_i loop re-runs
    the whole body `reps` times with the collectives elided (collectives
    cannot sit inside control flow) so device time dominates wall clock.
    """
    import concourse.bacc as bacc
    import concourse.mybir as mybir
    from concourse import tile
    from concourse._compat import get_trn_type

    dt = mybir.dt
    KL = J_lo + KO_lo
    KH = J_hi + KO_hi
    K = KL + KH
    KO = KO_lo + KO_hi
    NCH = TPC * K
    NW = NCH * 8

    nc = bacc.Bacc(
        get_trn_type() or "TRN2",
        target_bir_lowering=False,
        debug=False,
        enable_asserts=False,
        num_devices=M_CORES,
        num_swdge_queues=nq,
    )

    # I/O (xT/W in bf16; dinv/dloc fp32)
    xT_p = nc.dram_tensor("xT", [128, NPAD], dt.bfloat16, kind="ExternalInput")
    W1_p = nc.dram_tensor("W1", [IN_C, HID_C], dt.bfloat16, kind="ExternalInput")
    W2_p = nc.dram_tensor("W2", [HID_C, HID_C], dt.bfloat16, kind="ExternalInput")
    W3_p = nc.dram_tensor("W3", [HID_C, OUT_C], dt.bfloat16, kind="ExternalInput")
    dinv_p = nc.dram_tensor("dinv", [128, TPC], dt.float32, kind="ExternalInput")
    idx_p = nc.dram_tensor("idx", [128, NW], dt.int16, kind="ExternalInput")
    ident_p = nc.dram_tensor("ident", [128, 128], dt.bfloat16, kind="ExternalInput")
    oh_dt = dt.float8e4
    ohtab_p = nc.dram_tensor("ohtab", [128, TPC * KO * 128], oh_dt, kind="ExternalInput")
    if with_bias:
        b1_p = nc.dram_tensor("b1r", [128, HID_C], dt.float32, kind="ExternalInput")
        b2_p = nc.dram_tensor("b2r", [128, HID_C], dt.float32, kind="ExternalInput")
        b3_p = nc.dram_tensor("b3r", [128, OUT_C], dt.float32, kind="ExternalInput")
    out_p = nc.dram_tensor("out", [NPAD, OUT_C], dt.float32, kind="ExternalOutput")

    RG = [list(range(M_CORES))]
    AF = mybir.ActivationFunctionType
    OP = mybir.AluOpType

    with tile.TileContext(nc) as tc, tc.tile_pool(name="persist", bufs=1) as pp:
        # persistent SBUF tiles (one slot each)
        hT_a = pp.tile([128, NPAD], dt.bfloat16, name="hT_a")
        hT_b = pp.tile([128, NPAD], dt.bfloat16, name="hT_b")
        w1_sb = pp.tile([128, HID_C], dt.bfloat16, name="w1_sb")
        w2_sb = pp.tile([128, HID_C], dt.bfloat16, name="w2_sb")
        w3_sb = pp.tile([128, OUT_C], dt.bfloat16, name="w3_sb")
        dinv_sb = pp.tile([128, TPC], dt.float32, name="dinv_sb")
        idx_sb = pp.tile([128, NW], dt.int16, name="idx_sb")
        ident_sb = pp.tile([128, 128], dt.bfloat16, name="ident_sb")
        bias_sb = []

        nc.sync.dma_start(hT_a[:], xT_p[:])
        nc.sync.dma_start(w1_sb[:], W1_p[:])
        nc.sync.dma_start(w2_sb[:], W2_p[:])
        nc.sync.dma_start(w3_sb[:], W3_p[:])
        nc.sync.dma_start(dinv_sb[:], dinv_p[:])
        nc.sync.dma_start(idx_sb[:], idx_p[:])
        nc.sync.dma_start(ident_sb[:], ident_p[:])
        if with_bias:
            for p, cc in ((b1_p, HID_C), (b2_p, HID_C), (b3_p, OUT_C)):
                t = pp.tile([128, cc], dt.float32, name=f"bias{len(bias_sb)}_sb")
                nc.sync.dma_start(t[:], p[:])
                bias_sb.append(t)

        layers = [
            (w1_sb, HID_C, True, hT_a, hT_b),
            (w2_sb, HID_C, True, hT_b, hT_a),
            (w3_sb, OUT_C, False, hT_a, None),
        ]

        with (
            tc.tile_pool(name="gsb", bufs=2) as gsb_pool,
            tc.tile_pool(name="msg", bufs=msg_bufs) as msg_pool,
            tc.tile_pool(name="ohs", bufs=3) as ohs_pool,
            tc.tile_pool(name="eps", bufs=3) as eps_pool,
            tc.tile_pool(name="psA", bufs=2, space="PSUM") as psA_pool,
            tc.tile_pool(name="psS", bufs=2, space="PSUM") as psS_pool,
            tc.tile_pool(name="psT", bufs=2, space="PSUM") as psT_pool,
            tc.tile_pool(name="dram", bufs=1, space="DRAM") as dram_pool,
        ):
            # tables are bf16, layer-3 table padded to 128 cols (upper half
            # junk, never read by compute: matmul rhs slices cols 0:C)
            glocs = [
                dram_pool.tile([NPAD, 128], dt.bfloat16, name=f"gloc{i}")
                for i in range(3)
            ]
            gfuls = [
                dram_pool.tile(
                    [TBL, 128], dt.bfloat16,
                    addr_space="Shared" if M_CORES > 4 else "Local",
                    name=f"gful{i}",
                )
                for i in range(3)
            ]

            def emit_layers(with_cc):
                for li, (w_sb, C, relu, hT_in, hT_out) in enumerate(layers):
                    gloc, gful = glocs[li], gfuls[li]

                    # stage A: g = dinv * (h @ W) for own nodes
                    g_sb = gsb_pool.tile([128, TPC, 128], dt.bfloat16, tag="gsb")
                    for t in range(TPC):
                        psA = psA_pool.tile([128, C], dt.float32, tag="psA")
                        nc.tensor.matmul(
                            psA[:],
                            lhsT=hT_in[:, t * 128 : (t + 1) * 128],
                            rhs=w_sb[:, :C],
                            start=True,
                            stop=True,
                        )
                        nc.vector.tensor_scalar_mul(
                            g_sb[:, t, :C], psA[:], dinv_sb[:, t : t + 1]
                        )
                    nc.sync.dma_start(
                        gloc[:].rearrange("(t p) c -> p t c", p=128), g_sb[:]
                    )

                    # stage B: replicate the g table
                    if with_cc:
                        nc.gpsimd.collective_compute(
                            "AllGather",
                            OP.bypass,
                            replica_groups=RG,
                            ins=[gloc[:]],
                            outs=[gful[:]],
                        )
                    g_lo = gful[0 : min(LO_CAP, TBL), :]
                    g_hi = gful[HI_OFF:TBL, :]

                    # stage C: merged gather per block + one-hot scatter per
                    # destination tile
                    qctr = [0]

                    def gather_run(msg, boff, c0, c1, gview):
                        """One dma_gather over chunk run [c0, c1) of the block."""
                        n = (c1 - c0) * 128
                        nc.gpsimd.dma_gather(
                            msg[:, c0:c1, :],
                            gview[:],
                            idx_sb[:, (boff + c0) * 8 : (boff + c1) * 8],
                            n,
                            n,
                            128,
                            elem_step=128,
                            single_packet=spk > 0,
                            queue_num=qctr[0] % nq,
                        )
                        qctr[0] += 1

                    for b in range(NBLK):
                        boff = b * NTB * K
                        if 'nogather' not in abl:
                            msg = msg_pool.tile(
                                [128, NTB * K, 128], dt.bfloat16, tag="msg"
                            )
                            step = spk if spk > 0 else 10**9
                            for (s0, s1, gv) in (
                                (0, NTB * KL, g_lo),
                                (NTB * KL, NTB * K, g_hi),
                            ):
                                for c0 in range(s0, s1, step):
                                    gather_run(msg, boff, c0, min(c0 + step, s1), gv)
                        if KO and 'nooh' not in abl:
                            # block-level fp8 one-hot stream on the ACT HWDGE
                            # queue (keeps SP queue and SWDGE rings clear)
                            oh_b = ohs_pool.tile(
                                [128, NTB * KO, 128], oh_dt, tag="ohs"
                            )
                            nc.scalar.dma_start(
                                oh_b[:],
                                ohtab_p[
                                    :,
                                    b * NTB * KO * 128 : (b + 1) * NTB * KO * 128,
                                ].rearrange("p (k c) -> p k c", k=NTB * KO),
                            )
                        for tloc in range(NTB):
                            t = b * NTB + tloc
                            psS = psS_pool.tile([128, C], dt.float32, tag="psS")
                            # (msg chunk within block, overflow one-hot idx or None)
                            chunks = (
                                [(tloc * KL + j, None) for j in range(J_lo)]
                                + [(tloc * KL + J_lo + j, j) for j in range(KO_lo)]
                                + [(NTB * KL + tloc * KH + j, None) for j in range(J_hi)]
                                + [(NTB * KL + tloc * KH + J_hi + j, KO_lo + j)
                                   for j in range(KO_hi)]
                            )
                            if 'nomm' in abl:
                                chunks = chunks[:1]
                            for jj, (mc, ko) in enumerate(chunks):
                                if ko is None or 'nooh' in abl:
                                    mm_lhs = ident_sb[:]
                                else:
                                    mm_lhs = oh_b[:, tloc * KO + ko, :]
                                mm_rhs = (
                                    msg[:, mc, :C]
                                    if 'nogather' not in abl
                                    else hT_a[:, jj * 64 : jj * 64 + C]
                                )
                                nc.tensor.matmul(
                                    psS[:],
                                    lhsT=mm_lhs,
                                    rhs=mm_rhs,
                                    start=(jj == 0),
                                    stop=(jj == len(chunks) - 1),
                                )
                            # epilogue: + self-loop, * dinv, (+bias), relu
                            acc = eps_pool.tile([128, C], dt.float32, tag="acc")
                            nc.vector.tensor_add(
                                acc[:], psS[:], g_sb[:, t, :C]
                            )
                            h_t = eps_pool.tile(
                                [128, C],
                                dt.bfloat16 if hT_out is not None else dt.float32,
                                tag="h_t",
                            )
                            if with_bias:
                                nc.vector.tensor_scalar_mul(
                                    acc[:], acc[:], dinv_sb[:, t : t + 1]
                                )
                                nc.vector.tensor_add(acc[:], acc[:], bias_sb[li][:])
                                if relu:
                                    nc.scalar.activation(h_t[:], acc[:], AF.Relu)
                                else:
                                    nc.scalar.copy(h_t[:], acc[:])
                            else:
                                if relu:
                                    nc.scalar.activation(
                                        h_t[:], acc[:], AF.Relu,
                                        scale=dinv_sb[:, t : t + 1],
                                    )
                                else:
                                    nc.scalar.mul(
                                        h_t[:], acc[:], dinv_sb[:, t : t + 1]
                                    )
                            if hT_out is not None:
                                psT = psT_pool.tile([128, 128], dt.bfloat16, tag="psT")
                                nc.tensor.transpose(psT[:], h_t[:], ident_sb[:])
                                nc.vector.tensor_copy(
                                    hT_out[:, t * 128 : (t + 1) * 128], psT[:]
                                )
                            else:
                                nc.sync.dma_start(
                                    out_p[t * 128 : (t + 1) * 128, :], h_t[:]
                                )

            emit_layers(with_cc=True)
            if reps:
                with tc.For_i(0, reps, 1):
                    emit_layers(with_cc=False)

    nc.compile()
    return nc


_NC_CACHE: dict = {}


def _get_nc(J_lo, J_hi, KO_lo, KO_hi, with_bias):
    key = (J_lo, J_hi, KO_lo, KO_hi, with_bias)
    if key not in _NC_CACHE:
        _NC_CACHE[key] = _build_nc(*key)
    return _NC_CACHE[key]


# ---------------------------------------------------------------------------
# entry point
# ---------------------------------------------------------------------------
def _prepare(x, edge_index, W1, b1, W2, b2, W3, b3):
    import ml_dtypes

    bf16 = ml_dtypes.bfloat16
    x = np.asarray(x, dtype=np.float32)
    W1 = np.asarray(W1, dtype=np.float32)
    W2 = np.asarray(W2, dtype=np.float32)
    W3 = np.asarray(W3, dtype=np.float32)
    b1 = np.asarray(b1, dtype=np.float32)
    b2 = np.asarray(b2, dtype=np.float32)
    b3 = np.asarray(b3, dtype=np.float32)

    plan = _plan(np.asarray(edge_index))
    with_bias = bool(np.any(b1) or np.any(b2) or np.any(b3))
    nc = _get_nc(
        plan["J_lo"], plan["J_hi"], plan["KO_lo"], plan["KO_hi"], with_bias
    )

    fp8 = ml_dtypes.float8_e4m3
    ident = np.eye(128, dtype=np.float32).astype(bf16)
    pos_local = plan["pos_local"]

    in_maps = []
    for k in range(M_CORES):
        pc = plan["per_core"][k]
        v0 = k * NPC
        xT = np.zeros((128, NPAD), dtype=np.float32)
        xT[:, pos_local[v0 : v0 + NPC]] = x[v0 : v0 + NPC].T
        im = dict(
            xT=xT.astype(bf16),
            W1=W1.astype(bf16), W2=W2.astype(bf16), W3=W3.astype(bf16),
            dinv=pc["dinv"], idx=pc["idx"],
            ident=ident, ohtab=pc["ohtab"].astype(fp8),
        )
        if with_bias:
            im["b1r"] = np.tile(b1, (128, 1)).astype(np.float32)
            im["b2r"] = np.tile(b2, (128, 1)).astype(np.float32)
            im["b3r"] = np.tile(b3, (128, 1)).astype(np.float32)
        in_maps.append(im)

    def unpermute(results):
        out = np.empty((N_NODES, OUT_C), dtype=np.float32)
        for k in range(M_CORES):
            v0 = k * NPC
            r = results[k]["out"]
            out[v0 : v0 + NPC] = r[pos_local[v0 : v0 + NPC]]
        return out

    return nc, in_maps, unpermute


def kernel(x, edge_index, W1, b1, W2, b2, W3, b3):
    from concourse.bass_utils import run_bass_kernel_spmd

    nc, in_maps, unpermute = _prepare(x, edge_index, W1, b1, W2, b2, W3, b3)
    res = run_bass_kernel_spmd(nc, in_maps, list(range(M_CORES)))
    return unpermute(res.results)
